# revision 21
# baseline (speedup 1.0000x reference)
"""AttnBlock (GroupNorm -> QKV 1x1 -> full attention over 1024 tokens -> out-proj
+ residual) for x [32, 512, 32, 32] f32, distributed data-parallel over 8
NeuronCores (4 samples per core, weights replicated).

Per-core single-NC Bass/Tile kernel. fp8(e4m3) DoubleRow TensorE compute for
the four 1x1 projections and the score matmul (2 contraction rows per PE
cell -> ~1.5x bf16 throughput); bf16 for the PV matmul (softmax-weight fp8
quantization dominates the error budget, so est stays bf16); f32 softmax
stats.

  - GroupNorm via per-channel bn_stats/bn_aggr + tiny selector matmuls for the
    cross-partition group reduce (fp32), fast-inverse-sqrt on DVE.
  - h, q, k stored fp8; scores computed TRANSPOSED (ST[j,i] = sum_d K[d,j]
    Q[d,i]) so P~ = exp(ST*c) has the contraction axis j on partitions and PV
    needs no transposes. V computed transposed ([hw, d], bf16 out).
  - Softmax denominator: DVE pairwise-add tree over the 8 exp tiles in bf16
    (2x DVE rate), then one (1/16)-matrix bf16 stationary matmul that reduces
    the remaining 128 partitions and replicates den/16 across partitions;
    rep = 16/den via reciprocal_approx_fast. The PV psum->sbuf copy multiplies
    by rep, so o is stored as fp8 at 16x scale (lands in e4m3 normal range);
    the final residual op multiplies the out-proj psum by 1/16.
  - Cross-sample software pipeline: TensorE order per iteration b is
    [scores(b) | QK(b+1) | PV(b) | V(b+1) | proj(b)] so the ACT exp stream
    (the slowest per-phase engine, ~1.1us/tile vs 0.75us/tile for the fp8
    score matmuls) of sample b is hidden behind the QK matmuls of sample b+1.
    GroupNorm for sample b+2 runs on DVE during iteration b. The psum->sbuf
    copies are split across engines (Q on DVE, K and V on ACT after the exp
    stream drains; GPSIMD cannot read PSUM) so the psq pool never backs up
    into the TensorE stream.
"""

import os
import sys

import numpy as np

sys.path.insert(0, "/opt/trn_rl_repo")

import ml_dtypes  # noqa: E402

import concourse.bass as bass  # noqa: E402
import concourse.tile as tile  # noqa: E402
from concourse import bacc, mybir  # noqa: E402

P = 128
B_FULL, C, H, W = 32, 512, 32, 32
HW = H * W            # 1024 tokens
N_CORES = 8
NB = B_FULL // N_CORES  # 4 samples per core
NT = C // P           # 4 channel tiles
NP = NT // 2          # 2 DoubleRow channel-tile pairs
NJ = HW // P          # 8 token tiles
NGROUPS = 32
GS = C // NGROUPS     # 16 channels per group
G_PER_TILE = P // GS  # 8 groups per 128-channel tile
EPS = 1e-6
CINV = float(C) ** -0.5
OSC = 16.0            # fp8 o is stored at 16x scale

f32 = mybir.dt.float32
bf16 = mybir.dt.bfloat16
f8 = mybir.dt.float8e4
ALU = mybir.AluOpType
ACT = mybir.ActivationFunctionType
DR = mybir.MatmulPerfMode.DoubleRow


def build_nc(zero_qk_bias=False, zero_v_bias=False, zero_t_bias=False):
    """Build the single-core Bass graph (SPMD: same graph on all 8 cores).

    zero_*_bias: when the corresponding biases are all-zero (true for this
    problem's setup_inputs), the psum->sbuf copies drop the bias add.
    """
    nc = bacc.Bacc("TRN2", target_bir_lowering=False, debug=False)

    x_d = nc.dram_tensor("x", [NB, C, HW], f32, kind="ExternalInput")
    wq_d = nc.dram_tensor("wq", [P, NT, C], f8, kind="ExternalInput")
    wk_d = nc.dram_tensor("wk", [P, NT, C], f8, kind="ExternalInput")
    wv_d = nc.dram_tensor("wv", [P, NT, C], f8, kind="ExternalInput")
    wt_d = nc.dram_tensor("wt", [P, NT, C], f8, kind="ExternalInput")
    # packed per-partition bias columns: [:, 0, :]=bq, [:, 1, :]=bk, [:, 2, :]=bt
    bqkt_d = nc.dram_tensor("bqkt", [P, 3, NT], f32, kind="ExternalInput")
    bv_d = nc.dram_tensor("bv_rep", [P, C], f32, kind="ExternalInput")
    # gn affine columns: [:, 0, :]=gamma, [:, 1, :]=beta
    gab_d = nc.dram_tensor("gn_ab", [P, 2, NT], f32, kind="ExternalInput")
    # block-diagonal group-average matrix: GG[k,p] = 1/16 iff k//16 == p//16
    gg_d = nc.dram_tensor("gg", [P, P], f32, kind="ExternalInput")
    out_d = nc.dram_tensor("out", [NB, C, HW], f32, kind="ExternalOutput")

    with tile.TileContext(nc) as tc:
        with (
            tc.tile_pool(name="consts", bufs=1) as consts,
            tc.tile_pool(name="hp", bufs=2) as hp,
            tc.tile_pool(name="qkp", bufs=1) as qkp,
            tc.tile_pool(name="vtp", bufs=2) as vtp,
            tc.tile_pool(name="est", bufs=1) as estp,
            tc.tile_pool(name="op", bufs=1) as op,
            tc.tile_pool(name="outp", bufs=3) as outp,
            tc.tile_pool(name="small", bufs=2) as small,
            tc.tile_pool(name="sump", bufs=1) as sump,
            tc.tile_pool(name="psb", bufs=2, space="PSUM") as psb,
            tc.tile_pool(name="psq", bufs=4, space="PSUM") as psq,
        ):
            # ---- x[0] gates everything: one tile per DMA queue (4 parallel
            #      rings), then weights/consts sequenced by first-use time.
            x_sb = consts.tile([P, NB, NT, HW], f32, tag="x")
            wq_sb = consts.tile([P, NT, C], f8, tag="wq")
            wk_sb = consts.tile([P, NT, C], f8, tag="wk")
            wv_sb = consts.tile([P, NT, C], f8, tag="wv")
            wt_sb = consts.tile([P, NT, C], f8, tag="wt")
            gab_sb = consts.tile([P, 2, NT], f32, tag="gab")
            gg_sb = consts.tile([P, P], f32, tag="gg")
            bqkt_sb = consts.tile([P, 3, NT], f32, tag="bqkt")
            bv_sb = consts.tile([P, C], f32, tag="bv")

            # x[0] quarters round-robin across the 3 DMA queues (x[0] wire
            # time gates the GN(0) -> QK(0) chain); gg+gab first on gpsimd
            # (tiny, needed by affine(0)).
            nc.gpsimd.dma_start(out=gg_sb[:, :], in_=gg_d[:, :])
            nc.gpsimd.dma_start(out=gab_sb[:, :, :], in_=gab_d[:, :, :])
            engs = (nc.sync, nc.scalar, nc.gpsimd)
            qi = 0
            for t in range(NT):
                for h0 in (0, 256, 512, 768):
                    engs[qi % 3].dma_start(
                        out=x_sb[:, 0, t, h0:h0 + 256],
                        in_=x_d[0, t * P:(t + 1) * P, h0:h0 + 256])
                    qi += 1
            # weights right behind x[0] (land ~when QK(0)/V(0) start)
            nc.sync.dma_start(out=wq_sb[:, :, :], in_=wq_d[:, :, :])
            nc.scalar.dma_start(out=wk_sb[:, :, :], in_=wk_d[:, :, :])
            nc.gpsimd.dma_start(out=wv_sb[:, :, :], in_=wv_d[:, :, :])
            # x[1] split sync/scalar (needed by stats(1) mid-prologue)
            for t in (0, 1):
                nc.scalar.dma_start(out=x_sb[:, 1, t, :],
                                    in_=x_d[1, t * P:(t + 1) * P, :])
            for t in (2, 3):
                nc.sync.dma_start(out=x_sb[:, 1, t, :],
                                  in_=x_d[1, t * P:(t + 1) * P, :])
            # x[3] on sync (needed iteration 1)
            for t in range(NT):
                nc.sync.dma_start(out=x_sb[:, 3, t, :],
                                  in_=x_d[3, t * P:(t + 1) * P, :])
            # gpsimd: remaining consts + x[2] + wt
            nc.gpsimd.dma_start(out=bqkt_sb[:, :, :], in_=bqkt_d[:, :, :])
            for t in range(NT):
                nc.gpsimd.dma_start(out=x_sb[:, 2, t, :],
                                    in_=x_d[2, t * P:(t + 1) * P, :])
            nc.gpsimd.dma_start(out=bv_sb[:, :], in_=bv_d[:, :])
            nc.gpsimd.dma_start(out=wt_sb[:, :, :], in_=wt_d[:, :, :])
            # (1/16)-matrix: reduces partitions AND folds den/16 for the
            # 16x fp8 o-scale
            ones_sb = consts.tile([P, P], bf16, tag="ones")
            nc.vector.memset(ones_sb[:, :], 1.0 / OSC)
            magic_sb = consts.tile([P, NT], mybir.dt.int32, tag="magic")
            nc.vector.memset(magic_sb[:, :], 0x5F3759DF)
            # dummy Exp: pulls the ACT Exp-table load into the idle prologue
            # (off the first real exp's critical path)
            expwarm = small.tile([P, 1], f32, tag="expwarm")
            nc.scalar.activation(out=expwarm[:, :], in_=ones_sb[:, 0:1],
                                 func=ACT.Exp)

            a_all = consts.tile([P, NB, NT], f32, tag="a_all")
            b_all = consts.tile([P, NB, NT], f32, tag="b_all")

            # PE warm-up: harmless fp32 matmuls on the earliest-arriving x
            # tile so the HAM clock-gate is released before the real stream.
            warm_ps = psq.tile([P, 512], f32, tag="qkv")
            for w in range(5):
                nc.tensor.matmul(
                    warm_ps[:, :], x_sb[:, 0, 0, 0:128], x_sb[:, 0, 0, 0:512],
                    start=(w == 0), stop=(w == 4),
                )

            def gn_stats(b, after=None):
                """bn stats -> per-channel (mean, Ex2) packed in mv."""
                mv = small.tile([P, NT, 2], f32, tag="mv")
                nsub = 2
                step = HW // nsub
                for t in range(NT):
                    st6 = small.tile([P, nsub, 6], f32, tag="st6")
                    for q in range(nsub):
                        iq = nc.vector.bn_stats(
                            out=st6[:, q, :],
                            in_=x_sb[:, b, t, q * step:(q + 1) * step])
                        if after is not None:
                            tile.add_dep_helper(iq.ins, after.ins, sync=False,
                                                reason="gn stats after prev apply")
                    nc.vector.bn_aggr(out=mv[:, t, :], in_=st6[:, :, :])
                msq = small.tile([P, NT], f32, tag="msq")
                nc.vector.tensor_mul(msq[:, :], mv[:, :, 0], mv[:, :, 0])
                nc.vector.tensor_add(mv[:, :, 1], mv[:, :, 1], msq[:, :])
                return mv

            def gn_affine(b, mv, use_act_sqrt=False):
                """fused group-avg+broadcast matmul, then form per-channel A/B."""
                bc_ps = psq.tile([P, 512], f32, tag="qkv")
                nc.tensor.matmul(bc_ps[:, :NT * 2], gg_sb[:, :], mv[:, :, :],
                                 start=True, stop=True)
                bc = small.tile([P, NT, 2], f32, tag="bcs")
                nc.vector.tensor_copy(bc[:, :, :], bc_ps[:, 0:NT * 2])
                vb = small.tile([P, NT], f32, tag="vb")
                nc.vector.tensor_mul(vb[:, :], bc[:, :, 0], bc[:, :, 0])
                nc.vector.tensor_sub(vb[:, :], bc[:, :, 1], vb[:, :])
                if use_act_sqrt:
                    # sample 0 (pre-exp): the shorter ACT chain wins and its
                    # Sqrt table load cannot evict a not-yet-loaded Exp table
                    nc.vector.tensor_scalar_add(vb[:, :], vb[:, :], EPS)
                    nc.scalar.sqrt(vb[:, :], vb[:, :])
                    nc.vector.reciprocal(vb[:, :], vb[:, :])
                    y1 = vb
                else:
                    nc.vector.tensor_scalar_add(vb[:, :], vb[:, :], EPS)
                    # rstd = rsqrt(var+eps): fast-inverse-sqrt + 2 Newton steps
                    # (all-DVE: keeps Sqrt off ACT so it never evicts Exp)
                    ii = small.tile([P, NT], mybir.dt.int32, tag="ii")
                    nc.vector.tensor_scalar(
                        out=ii[:, :], in0=vb.bitcast(mybir.dt.int32)[:, :],
                        scalar1=1, scalar2=None, op0=ALU.arith_shift_right)
                    nc.vector.tensor_tensor(ii[:, :], magic_sb[:, :], ii[:, :],
                                            op=ALU.subtract)
                    y0 = ii.bitcast(f32)
                    yt = small.tile([P, NT], f32, tag="yt")
                    y1 = small.tile([P, NT], f32, tag="y1")
                    nc.vector.tensor_mul(yt[:, :], vb[:, :], y0[:, :])
                    nc.vector.tensor_mul(yt[:, :], yt[:, :], y0[:, :])
                    nc.vector.tensor_scalar(out=yt[:, :], in0=yt[:, :], scalar1=-0.5,
                                            scalar2=1.5, op0=ALU.mult, op1=ALU.add)
                    nc.vector.tensor_mul(y1[:, :], y0[:, :], yt[:, :])
                    nc.vector.tensor_mul(yt[:, :], vb[:, :], y1[:, :])
                    nc.vector.tensor_mul(yt[:, :], yt[:, :], y1[:, :])
                    nc.vector.tensor_scalar(out=yt[:, :], in0=yt[:, :], scalar1=-0.5,
                                            scalar2=1.5, op0=ALU.mult, op1=ALU.add)
                    nc.vector.tensor_mul(y1[:, :], y1[:, :], yt[:, :])
                tmp = small.tile([P, NT], f32, tag="tmpab")
                nc.vector.tensor_mul(a_all[:, b, :], y1[:, :], gab_sb[:, 0, :])
                nc.vector.tensor_mul(tmp[:, :], bc[:, :, 0], a_all[:, b, :])
                nc.vector.tensor_sub(b_all[:, b, :], gab_sb[:, 1, :], tmp[:, :])

            def apply_h(b, after=None):
                """h = x*A + B (fp8)"""
                h = hp.tile([P, NT, HW], f8, tag="h")
                last = None
                for t in range(NT):
                    last = nc.vector.tensor_scalar(
                        out=h[:, t, :], in0=x_sb[:, b, t, :],
                        scalar1=a_all[:, b, t:t + 1], scalar2=b_all[:, b, t:t + 1],
                        op0=ALU.mult, op1=ALU.add,
                    )
                    if after is not None:
                        tile.add_dep_helper(last.ins, after.ins, sync=False,
                                            reason="apply after O-scales")
                return h, last

            def qk(b, h_sb):
                """Q,K fp8 [d, hw] via DoubleRow fp8 matmuls; psum->sbuf
                copies on DVE."""
                q_sb = qkp.tile([P, NT, HW], f8, tag="q")
                k_sb = qkp.tile([P, NT, HW], f8, tag="k")
                # Q psum->sbuf copies on DVE, K copies on ACT: split across
                # engines so the psq pool drains at 2x copy rate and neither
                # engine's queue backs up.
                for dst_sb, w_sb, bias_idx, eng in (
                        (q_sb, wq_sb, 0, "dve"), (k_sb, wk_sb, 1, "act")):
                    for dm in range(NT):
                        ps0 = psq.tile([P, 512], f32, tag="qkv")
                        ps1 = psq.tile([P, 512], f32, tag="qkv")
                        for kp in range(NP):
                            lhsT = w_sb[:, 2 * kp:2 * kp + 2, dm * P:(dm + 1) * P]
                            st, sp = (kp == 0), (kp == NP - 1)
                            nc.tensor.matmul(ps0[:, :], lhsT,
                                             h_sb[:, 2 * kp:2 * kp + 2, 0:512],
                                             start=st, stop=sp, perf_mode=DR)
                            nc.tensor.matmul(ps1[:, :], lhsT,
                                             h_sb[:, 2 * kp:2 * kp + 2, 512:1024],
                                             start=st, stop=sp, perf_mode=DR)
                        for ps, nsl in ((ps0, slice(0, 512)), (ps1, slice(512, 1024))):
                            if zero_qk_bias:
                                if eng == "act":
                                    nc.scalar.copy(dst_sb[:, dm, nsl], ps[:, :])
                                else:
                                    nc.vector.tensor_copy(dst_sb[:, dm, nsl], ps[:, :])
                            else:
                                if eng == "act":
                                    nc.scalar.add(dst_sb[:, dm, nsl], ps[:, :],
                                                  bqkt_sb[:, bias_idx, dm:dm + 1])
                                else:
                                    nc.vector.tensor_scalar(
                                        out=dst_sb[:, dm, nsl], in0=ps[:, :],
                                        scalar1=bqkt_sb[:, bias_idx, dm:dm + 1],
                                        scalar2=None, op0=ALU.add)
                return q_sb, k_sb

            def vmm(b, h_sb):
                """V bf16 transposed [hw, d]; psum->sbuf copies on ACT."""
                vt_sb = vtp.tile([P, NJ, C], bf16, tag="vt")
                for jm in range(NJ):
                    ps = psq.tile([P, 512], f32, tag="qkv")
                    for kp in range(NP):
                        nc.tensor.matmul(
                            ps[:, :],
                            h_sb[:, 2 * kp:2 * kp + 2, jm * P:(jm + 1) * P],
                            wv_sb[:, 2 * kp:2 * kp + 2, :],
                            start=(kp == 0), stop=(kp == NP - 1), perf_mode=DR,
                        )
                    if zero_v_bias:
                        nc.scalar.copy(vt_sb[:, jm, :], ps[:, :])
                    else:
                        nc.vector.tensor_add(vt_sb[:, jm, :], ps[:, :], bv_sb[:, :])
                return vt_sb

            # ---------------- prologue: GN(0), qk(0), GN(1) ----------------
            # V(0) is NOT issued here: its ACT copies would queue ahead of
            # exp(0) and delay the whole pipeline. It is issued inside
            # iteration 0 (after the den section), mirroring the steady-state
            # vmm(b+1) position, so its copies run after exp(0) drains.
            mv0 = gn_stats(0)
            gn_affine(0, mv0)
            h0_sb, last_apply = apply_h(0)
            h_next = h0_sb

            qk_next = qk(0, h0_sb)

            mv1 = gn_stats(1, after=last_apply)
            gn_affine(1, mv1)
            h_next, last_apply = apply_h(1)

            vt_next = None

            # ---------------- main loop ----------------
            for b in range(NB):
                q_sb, k_sb = qk_next
                vt_sb = vt_next
                h_sb = h_next

                # ---- scores transposed + exp (ACT) ----
                est_sb = estp.tile([P, NJ, HW], bf16, tag="est")
                for jm in range(NJ):
                    st_ps = psb.tile([P, HW], f32, tag="big")
                    for kp in range(NP):
                        lhsT = k_sb[:, 2 * kp:2 * kp + 2, jm * P:(jm + 1) * P]
                        st, sp = (kp == 0), (kp == NP - 1)
                        nc.tensor.matmul(st_ps[:, 0:512], lhsT,
                                         q_sb[:, 2 * kp:2 * kp + 2, 0:512],
                                         start=st, stop=sp, perf_mode=DR)
                        nc.tensor.matmul(st_ps[:, 512:1024], lhsT,
                                         q_sb[:, 2 * kp:2 * kp + 2, 512:1024],
                                         start=st, stop=sp, perf_mode=DR)
                    nc.scalar.activation(
                        out=est_sb[:, jm, :], in_=st_ps[:, :], func=ACT.Exp,
                        scale=CINV,
                    )

                # ---- softmax denominator ----
                # pairwise bf16 tree over the 8 exp tiles (DVE 2x rate), then
                # one (1/16)-matrix bf16 matmul reduces the last 128
                # partitions AND replicates den/16 across partitions. Issued
                # ahead of qk(b+1) so recip/rep unblocks PV's O-scales early.
                s0 = sump.tile([P, HW], bf16, tag="s0")
                s1 = sump.tile([P, HW], bf16, tag="s1")
                s2 = sump.tile([P, HW], bf16, tag="s2")
                s3 = sump.tile([P, HW], bf16, tag="s3")
                nc.vector.tensor_add(s0[:, :], est_sb[:, 0, :], est_sb[:, 1, :])
                nc.vector.tensor_add(s1[:, :], est_sb[:, 2, :], est_sb[:, 3, :])
                nc.vector.tensor_add(s2[:, :], est_sb[:, 4, :], est_sb[:, 5, :])
                nc.vector.tensor_add(s3[:, :], est_sb[:, 6, :], est_sb[:, 7, :])
                nc.vector.tensor_add(s0[:, :], s0[:, :], s1[:, :])
                nc.vector.tensor_add(s2[:, :], s2[:, :], s3[:, :])
                nc.vector.tensor_add(s0[:, :], s0[:, :], s2[:, :])
                rs0 = psq.tile([P, 512], f32, tag="qkv")
                rs1 = psq.tile([P, 512], f32, tag="qkv")
                nc.tensor.matmul(rs0[:, :], ones_sb[:, :], s0[:, 0:512],
                                 start=True, stop=True)
                nc.tensor.matmul(rs1[:, :], ones_sb[:, :], s0[:, 512:1024],
                                 start=True, stop=True)
                rep = op.tile([P, HW], f32, tag="rep")
                nc.vector.reciprocal_approx_fast(out=rep[:, 0:512], in_=rs0[:, :])
                nc.vector.reciprocal_approx_fast(out=rep[:, 512:1024], in_=rs1[:, :])

                # sample 0's V, deferred out of the prologue (see above)
                if b == 0:
                    vt_sb = vmm(0, h0_sb)

                # ---- Q,K for sample b+1 fill TensorE while ACT exps ----
                if b + 1 < NB:
                    qk_next = qk(b + 1, h_sb)

                # ---- PV (bf16): O16[c, i] = (sum_j VT[j, c] * est[j, i]) * rep ----
                o_sb = op.tile([P, NT, HW], f8, tag="o")
                for cm in range(NT):
                    o_ps = psb.tile([P, HW], f32, tag="big")
                    for jm in range(NJ):
                        lhsT = vt_sb[:, jm, cm * P:(cm + 1) * P]
                        st, sp = (jm == 0), (jm == NJ - 1)
                        nc.tensor.matmul(o_ps[:, 0:512], lhsT,
                                         est_sb[:, jm, 0:512], start=st, stop=sp)
                        nc.tensor.matmul(o_ps[:, 512:1024], lhsT,
                                         est_sb[:, jm, 512:1024], start=st, stop=sp)
                    last_oscale = nc.vector.tensor_mul(
                        o_sb[:, cm, :], o_ps[:, :], rep[:, :])

                # ---- V for sample b+1 (between PV and proj: its ACT copies
                #      run right after exp(b) ends, before exp(b+1) needs ACT)
                if b + 1 < NB:
                    vt_next = vmm(b + 1, h_sb)

                # sample b+2's GN: stats are a low-priority DVE filler; the
                # affine's tiny PE matmul embeds in the stream; the DVE
                # applies are ordered behind this sample's O-scales.
                if b + 2 < NB:
                    mv_next = gn_stats(b + 2, after=last_apply)
                    gn_affine(b + 2, mv_next)
                    h_next, last_apply = apply_h(b + 2, after=last_oscale)

                # ---- out-proj (fp8 DR on 16x o) + 1/16 + residual ----
                for dm in range(NT):
                    p_ps = psb.tile([P, HW], f32, tag="big")
                    for kp in range(NP):
                        lhsT = wt_sb[:, 2 * kp:2 * kp + 2, dm * P:(dm + 1) * P]
                        st, sp = (kp == 0), (kp == NP - 1)
                        nc.tensor.matmul(p_ps[:, 0:512], lhsT,
                                         o_sb[:, 2 * kp:2 * kp + 2, 0:512],
                                         start=st, stop=sp, perf_mode=DR)
                        nc.tensor.matmul(p_ps[:, 512:1024], lhsT,
                                         o_sb[:, 2 * kp:2 * kp + 2, 512:1024],
                                         start=st, stop=sp, perf_mode=DR)
                    out_t = outp.tile([P, HW], f32, tag="out")
                    if b == NB - 1:
                        # quarter-split the final residuals so the last
                        # out-DMAs start as early as possible
                        qengs = (nc.sync, nc.gpsimd, nc.scalar, nc.sync)
                        for qq in range(4):
                            h0 = qq * 256
                            if zero_t_bias:
                                nc.vector.scalar_tensor_tensor(
                                    out=out_t[:, h0:h0 + 256],
                                    in0=p_ps[:, h0:h0 + 256],
                                    scalar=1.0 / OSC,
                                    in1=x_sb[:, b, dm, h0:h0 + 256],
                                    op0=ALU.mult, op1=ALU.add,
                                )
                            else:
                                nc.vector.tensor_scalar(
                                    out=out_t[:, h0:h0 + 256],
                                    in0=p_ps[:, h0:h0 + 256],
                                    scalar1=1.0 / OSC,
                                    scalar2=bqkt_sb[:, 2, dm:dm + 1],
                                    op0=ALU.mult, op1=ALU.add)
                                nc.vector.tensor_add(
                                    out_t[:, h0:h0 + 256], out_t[:, h0:h0 + 256],
                                    x_sb[:, b, dm, h0:h0 + 256])
                            qengs[qq].dma_start(
                                out=out_d[b, dm * P:(dm + 1) * P, h0:h0 + 256],
                                in_=out_t[:, h0:h0 + 256])
                    else:
                        if zero_t_bias:
                            nc.vector.scalar_tensor_tensor(
                                out=out_t[:, :], in0=p_ps[:, :],
                                scalar=1.0 / OSC, in1=x_sb[:, b, dm, :],
                                op0=ALU.mult, op1=ALU.add,
                            )
                        else:
                            nc.vector.tensor_scalar(
                                out=out_t[:, :], in0=p_ps[:, :],
                                scalar1=1.0 / OSC,
                                scalar2=bqkt_sb[:, 2, dm:dm + 1],
                                op0=ALU.mult, op1=ALU.add)
                            nc.vector.tensor_add(
                                out_t[:, :], out_t[:, :], x_sb[:, b, dm, :])
                        nc.sync.dma_start(
                            out=out_d[b, dm * P:(dm + 1) * P, 0:512],
                            in_=out_t[:, 0:512])
                        nc.gpsimd.dma_start(
                            out=out_d[b, dm * P:(dm + 1) * P, 512:1024],
                            in_=out_t[:, 512:1024])

    nc.compile()
    return nc


def prep_inputs(inputs):
    """Host-side prep: per-core in_maps with pre-laid-out weights/constants."""
    e4 = ml_dtypes.float8_e4m3
    x = np.ascontiguousarray(np.asarray(inputs["x"], dtype=np.float32)).reshape(
        B_FULL, C, HW
    )

    def wprep(w):
        # [C, C] -> [P, NT, C]  (lhsT slices w[kc*128+p, d])
        return np.ascontiguousarray(
            np.asarray(w, dtype=np.float32).reshape(NT, P, C).transpose(1, 0, 2)
        ).astype(e4)

    def cols(v):
        # [C] -> [P, NT]
        return np.ascontiguousarray(
            np.asarray(v, dtype=np.float32).reshape(NT, P).T
        )

    bqkt = np.stack([cols(inputs["bq"]), cols(inputs["bk"]), cols(inputs["bt"])],
                    axis=1)  # [P, 3, NT]
    gab = np.stack([cols(inputs["gn_scale"]), cols(inputs["gn_bias"])], axis=1)
    bv_rep = np.tile(np.asarray(inputs["bv"], dtype=np.float32)[None, :], (P, 1))
    gg = np.zeros((P, P), np.float32)
    for p in range(P):
        gg[p, (p // GS) * GS:(p // GS + 1) * GS] = 1.0 / GS

    shared = {
        "wq": wprep(inputs["Wq"]), "wk": wprep(inputs["Wk"]),
        "wv": wprep(inputs["Wv"]), "wt": wprep(inputs["Wt"]),
        "bqkt": np.ascontiguousarray(bqkt), "bv_rep": bv_rep,
        "gn_ab": np.ascontiguousarray(gab), "gg": gg,
    }
    in_maps = []
    for c_id in range(N_CORES):
        m = dict(shared)
        m["x"] = np.ascontiguousarray(x[c_id * NB:(c_id + 1) * NB])
        in_maps.append(m)
    return in_maps


_NC_CACHE = {}


def get_nc(zero_qk_bias=True, zero_v_bias=True, zero_t_bias=True):
    key = (zero_qk_bias, zero_v_bias, zero_t_bias)
    if key not in _NC_CACHE:
        _NC_CACHE[key] = build_nc(zero_qk_bias=zero_qk_bias,
                                  zero_v_bias=zero_v_bias,
                                  zero_t_bias=zero_t_bias)
    return _NC_CACHE[key]


def run(inputs, trace=False):
    from concourse.bass_utils import run_bass_kernel_spmd

    zq = bool(
        np.all(np.asarray(inputs["bq"]) == 0) and np.all(np.asarray(inputs["bk"]) == 0)
    )
    zv = bool(np.all(np.asarray(inputs["bv"]) == 0))
    zt = bool(np.all(np.asarray(inputs["bt"]) == 0))
    nc = get_nc(zero_qk_bias=zq, zero_v_bias=zv, zero_t_bias=zt)
    in_maps = prep_inputs(inputs)
    res = run_bass_kernel_spmd(
        nc, in_maps, core_ids=list(range(N_CORES)), trace=trace
    )
    out = np.concatenate([np.asarray(r["out"]) for r in res.results], axis=0)
    return out.reshape(B_FULL, C, H, W), res


def kernel(**inputs):
    out, _ = run(inputs, trace=False)
    return out


# revision 22
# speedup vs baseline: 1.0081x; 1.0081x over previous
"""AttnBlock (GroupNorm -> QKV 1x1 -> full attention over 1024 tokens -> out-proj
+ residual) for x [32, 512, 32, 32] f32, distributed data-parallel over 8
NeuronCores (4 samples per core, weights replicated).

Per-core single-NC Bass/Tile kernel. fp8(e4m3) DoubleRow TensorE compute for
the four 1x1 projections and the score matmul (2 contraction rows per PE
cell -> ~1.5x bf16 throughput); bf16 for the PV matmul (softmax-weight fp8
quantization dominates the error budget, so est stays bf16); f32 softmax
stats.

  - GroupNorm via per-channel bn_stats/bn_aggr + tiny selector matmuls for the
    cross-partition group reduce (fp32), fast-inverse-sqrt on DVE.
  - h, q, k stored fp8; scores computed TRANSPOSED (ST[j,i] = sum_d K[d,j]
    Q[d,i]) so P~ = exp(ST*c) has the contraction axis j on partitions and PV
    needs no transposes. V computed transposed ([hw, d], bf16 out).
  - Softmax denominator: DVE pairwise-add tree over the 8 exp tiles in bf16
    (2x DVE rate), then one (1/16)-matrix bf16 stationary matmul that reduces
    the remaining 128 partitions and replicates den/16 across partitions;
    rep = 16/den via reciprocal_approx_fast. The PV psum->sbuf copy multiplies
    by rep, so o is stored as fp8 at 16x scale (lands in e4m3 normal range);
    the final residual op multiplies the out-proj psum by 1/16.
  - Cross-sample software pipeline: TensorE order per iteration b is
    [scores(b) | QK(b+1) | PV(b) | V(b+1) | proj(b)] so the ACT exp stream
    (the slowest per-phase engine, ~1.1us/tile vs 0.75us/tile for the fp8
    score matmuls) of sample b is hidden behind the QK matmuls of sample b+1.
    GroupNorm for sample b+2 runs on DVE during iteration b. The psum->sbuf
    copies are split across engines (Q on DVE, K and V on ACT after the exp
    stream drains; GPSIMD cannot read PSUM) so the psq pool never backs up
    into the TensorE stream.
"""

import os
import sys

import numpy as np

sys.path.insert(0, "/opt/trn_rl_repo")

import ml_dtypes  # noqa: E402

import concourse.bass as bass  # noqa: E402
import concourse.tile as tile  # noqa: E402
from concourse import bacc, mybir  # noqa: E402

P = 128
B_FULL, C, H, W = 32, 512, 32, 32
HW = H * W            # 1024 tokens
N_CORES = 8
NB = B_FULL // N_CORES  # 4 samples per core
NT = C // P           # 4 channel tiles
NP = NT // 2          # 2 DoubleRow channel-tile pairs
NJ = HW // P          # 8 token tiles
NGROUPS = 32
GS = C // NGROUPS     # 16 channels per group
G_PER_TILE = P // GS  # 8 groups per 128-channel tile
EPS = 1e-6
CINV = float(C) ** -0.5
OSC = 16.0            # fp8 o is stored at 16x scale

f32 = mybir.dt.float32
bf16 = mybir.dt.bfloat16
f8 = mybir.dt.float8e4
ALU = mybir.AluOpType
ACT = mybir.ActivationFunctionType
DR = mybir.MatmulPerfMode.DoubleRow


def build_nc(zero_qk_bias=False, zero_v_bias=False, zero_t_bias=False):
    """Build the single-core Bass graph (SPMD: same graph on all 8 cores).

    zero_*_bias: when the corresponding biases are all-zero (true for this
    problem's setup_inputs), the psum->sbuf copies drop the bias add.
    """
    nc = bacc.Bacc("TRN2", target_bir_lowering=False, debug=False)

    x_d = nc.dram_tensor("x", [NB, C, HW], f32, kind="ExternalInput")
    wq_d = nc.dram_tensor("wq", [P, NT, C], f8, kind="ExternalInput")
    wk_d = nc.dram_tensor("wk", [P, NT, C], f8, kind="ExternalInput")
    wv_d = nc.dram_tensor("wv", [P, NT, C], f8, kind="ExternalInput")
    wt_d = nc.dram_tensor("wt", [P, NT, C], f8, kind="ExternalInput")
    # packed per-partition bias columns: [:, 0, :]=bq, [:, 1, :]=bk, [:, 2, :]=bt
    bqkt_d = nc.dram_tensor("bqkt", [P, 3, NT], f32, kind="ExternalInput")
    bv_d = nc.dram_tensor("bv_rep", [P, C], f32, kind="ExternalInput")
    # gn affine columns: [:, 0, :]=gamma, [:, 1, :]=beta
    gab_d = nc.dram_tensor("gn_ab", [P, 2, NT], f32, kind="ExternalInput")
    # block-diagonal group-average matrix: GG[k,p] = 1/16 iff k//16 == p//16
    gg_d = nc.dram_tensor("gg", [P, P], f32, kind="ExternalInput")
    out_d = nc.dram_tensor("out", [NB, C, HW], f32, kind="ExternalOutput")

    with tile.TileContext(nc) as tc:
        with (
            tc.tile_pool(name="consts", bufs=1) as consts,
            tc.tile_pool(name="hp", bufs=2) as hp,
            tc.tile_pool(name="qkp", bufs=1) as qkp,
            tc.tile_pool(name="vtp", bufs=2) as vtp,
            tc.tile_pool(name="est", bufs=1) as estp,
            tc.tile_pool(name="op", bufs=1) as op,
            tc.tile_pool(name="outp", bufs=3) as outp,
            tc.tile_pool(name="small", bufs=2) as small,
            tc.tile_pool(name="sump", bufs=1) as sump,
            tc.tile_pool(name="psb", bufs=2, space="PSUM") as psb,
            tc.tile_pool(name="psq", bufs=4, space="PSUM") as psq,
        ):
            # ---- x[0] gates everything: one tile per DMA queue (4 parallel
            #      rings), then weights/consts sequenced by first-use time.
            x_sb = consts.tile([P, NB, NT, HW], f32, tag="x")
            wq_sb = consts.tile([P, NT, C], f8, tag="wq")
            wk_sb = consts.tile([P, NT, C], f8, tag="wk")
            wv_sb = consts.tile([P, NT, C], f8, tag="wv")
            wt_sb = consts.tile([P, NT, C], f8, tag="wt")
            gab_sb = consts.tile([P, 2, NT], f32, tag="gab")
            gg_sb = consts.tile([P, P], f32, tag="gg")
            bqkt_sb = consts.tile([P, 3, NT], f32, tag="bqkt")
            bv_sb = consts.tile([P, C], f32, tag="bv")

            # x[0] quarters round-robin across the 3 DMA queues (x[0] wire
            # time gates the GN(0) -> QK(0) chain); gg+gab first on gpsimd
            # (tiny, needed by affine(0)).
            nc.gpsimd.dma_start(out=gg_sb[:, :], in_=gg_d[:, :])
            nc.gpsimd.dma_start(out=gab_sb[:, :, :], in_=gab_d[:, :, :])
            engs = (nc.sync, nc.scalar, nc.gpsimd)
            qi = 0
            for t in range(NT):
                for h0 in (0, 256, 512, 768):
                    engs[qi % 3].dma_start(
                        out=x_sb[:, 0, t, h0:h0 + 256],
                        in_=x_d[0, t * P:(t + 1) * P, h0:h0 + 256])
                    qi += 1
            # weights right behind x[0] (land ~when QK(0)/V(0) start)
            nc.sync.dma_start(out=wq_sb[:, :, :], in_=wq_d[:, :, :])
            nc.scalar.dma_start(out=wk_sb[:, :, :], in_=wk_d[:, :, :])
            nc.gpsimd.dma_start(out=wv_sb[:, :, :], in_=wv_d[:, :, :])
            # x[1] split sync/scalar (needed by stats(1) mid-prologue)
            for t in (0, 1):
                nc.scalar.dma_start(out=x_sb[:, 1, t, :],
                                    in_=x_d[1, t * P:(t + 1) * P, :])
            for t in (2, 3):
                nc.sync.dma_start(out=x_sb[:, 1, t, :],
                                  in_=x_d[1, t * P:(t + 1) * P, :])
            # x[3] on sync (needed iteration 1)
            for t in range(NT):
                nc.sync.dma_start(out=x_sb[:, 3, t, :],
                                  in_=x_d[3, t * P:(t + 1) * P, :])
            # gpsimd: remaining consts + x[2] + wt
            nc.gpsimd.dma_start(out=bqkt_sb[:, :, :], in_=bqkt_d[:, :, :])
            for t in range(NT):
                nc.gpsimd.dma_start(out=x_sb[:, 2, t, :],
                                    in_=x_d[2, t * P:(t + 1) * P, :])
            nc.gpsimd.dma_start(out=bv_sb[:, :], in_=bv_d[:, :])
            nc.gpsimd.dma_start(out=wt_sb[:, :, :], in_=wt_d[:, :, :])
            # (1/16)-matrix: reduces partitions AND folds den/16 for the
            # 16x fp8 o-scale
            ones_sb = consts.tile([P, P], bf16, tag="ones")
            nc.vector.memset(ones_sb[:, :], 1.0 / OSC)
            magic_sb = consts.tile([P, NT], mybir.dt.int32, tag="magic")
            nc.vector.memset(magic_sb[:, :], 0x5F3759DF)
            # dummy Exp: pulls the ACT Exp-table load into the idle prologue
            # (off the first real exp's critical path)
            expwarm = small.tile([P, 1], f32, tag="expwarm")
            nc.scalar.activation(out=expwarm[:, :], in_=ones_sb[:, 0:1],
                                 func=ACT.Exp)

            a_all = consts.tile([P, NB, NT], f32, tag="a_all")
            b_all = consts.tile([P, NB, NT], f32, tag="b_all")

            # PE warm-up: harmless fp32 matmuls on the earliest-arriving x
            # tile so the HAM clock-gate is released before the real stream.
            warm_ps = psq.tile([P, 512], f32, tag="qkv")
            for w in range(5):
                nc.tensor.matmul(
                    warm_ps[:, :], x_sb[:, 0, 0, 0:128], x_sb[:, 0, 0, 0:512],
                    start=(w == 0), stop=(w == 4),
                )

            def gn_stats(b, after=None):
                """bn stats -> per-channel (mean, Ex2) packed in mv."""
                mv = small.tile([P, NT, 2], f32, tag="mv")
                nsub = 2
                step = HW // nsub
                for t in range(NT):
                    st6 = small.tile([P, nsub, 6], f32, tag="st6")
                    for q in range(nsub):
                        iq = nc.vector.bn_stats(
                            out=st6[:, q, :],
                            in_=x_sb[:, b, t, q * step:(q + 1) * step])
                        if after is not None:
                            tile.add_dep_helper(iq.ins, after.ins, sync=False,
                                                reason="gn stats after prev apply")
                    nc.vector.bn_aggr(out=mv[:, t, :], in_=st6[:, :, :])
                msq = small.tile([P, NT], f32, tag="msq")
                nc.vector.tensor_mul(msq[:, :], mv[:, :, 0], mv[:, :, 0])
                nc.vector.tensor_add(mv[:, :, 1], mv[:, :, 1], msq[:, :])
                return mv

            def gn_affine(b, mv, use_act_sqrt=False):
                """fused group-avg+broadcast matmul, then form per-channel A/B."""
                bc_ps = psq.tile([P, 512], f32, tag="qkv")
                nc.tensor.matmul(bc_ps[:, :NT * 2], gg_sb[:, :], mv[:, :, :],
                                 start=True, stop=True)
                bc = small.tile([P, NT, 2], f32, tag="bcs")
                nc.vector.tensor_copy(bc[:, :, :], bc_ps[:, 0:NT * 2])
                vb = small.tile([P, NT], f32, tag="vb")
                nc.vector.tensor_mul(vb[:, :], bc[:, :, 0], bc[:, :, 0])
                nc.vector.tensor_sub(vb[:, :], bc[:, :, 1], vb[:, :])
                if use_act_sqrt:
                    # sample 0 (pre-exp): the shorter ACT chain wins and its
                    # Sqrt table load cannot evict a not-yet-loaded Exp table
                    nc.vector.tensor_scalar_add(vb[:, :], vb[:, :], EPS)
                    nc.scalar.sqrt(vb[:, :], vb[:, :])
                    nc.vector.reciprocal(vb[:, :], vb[:, :])
                    y1 = vb
                else:
                    nc.vector.tensor_scalar_add(vb[:, :], vb[:, :], EPS)
                    # rstd = rsqrt(var+eps): fast-inverse-sqrt + 2 Newton steps
                    # (all-DVE: keeps Sqrt off ACT so it never evicts Exp)
                    ii = small.tile([P, NT], mybir.dt.int32, tag="ii")
                    nc.vector.tensor_scalar(
                        out=ii[:, :], in0=vb.bitcast(mybir.dt.int32)[:, :],
                        scalar1=1, scalar2=None, op0=ALU.arith_shift_right)
                    nc.vector.tensor_tensor(ii[:, :], magic_sb[:, :], ii[:, :],
                                            op=ALU.subtract)
                    y0 = ii.bitcast(f32)
                    yt = small.tile([P, NT], f32, tag="yt")
                    y1 = small.tile([P, NT], f32, tag="y1")
                    nc.vector.tensor_mul(yt[:, :], vb[:, :], y0[:, :])
                    nc.vector.tensor_mul(yt[:, :], yt[:, :], y0[:, :])
                    nc.vector.tensor_scalar(out=yt[:, :], in0=yt[:, :], scalar1=-0.5,
                                            scalar2=1.5, op0=ALU.mult, op1=ALU.add)
                    nc.vector.tensor_mul(y1[:, :], y0[:, :], yt[:, :])
                    nc.vector.tensor_mul(yt[:, :], vb[:, :], y1[:, :])
                    nc.vector.tensor_mul(yt[:, :], yt[:, :], y1[:, :])
                    nc.vector.tensor_scalar(out=yt[:, :], in0=yt[:, :], scalar1=-0.5,
                                            scalar2=1.5, op0=ALU.mult, op1=ALU.add)
                    nc.vector.tensor_mul(y1[:, :], y1[:, :], yt[:, :])
                tmp = small.tile([P, NT], f32, tag="tmpab")
                nc.vector.tensor_mul(a_all[:, b, :], y1[:, :], gab_sb[:, 0, :])
                nc.vector.tensor_mul(tmp[:, :], bc[:, :, 0], a_all[:, b, :])
                nc.vector.tensor_sub(b_all[:, b, :], gab_sb[:, 1, :], tmp[:, :])

            def apply_h(b, after=None):
                """h = x*A + B (fp8)"""
                h = hp.tile([P, NT, HW], f8, tag="h")
                last = None
                for t in range(NT):
                    last = nc.vector.tensor_scalar(
                        out=h[:, t, :], in0=x_sb[:, b, t, :],
                        scalar1=a_all[:, b, t:t + 1], scalar2=b_all[:, b, t:t + 1],
                        op0=ALU.mult, op1=ALU.add,
                    )
                    if after is not None:
                        tile.add_dep_helper(last.ins, after.ins, sync=False,
                                            reason="apply after O-scales")
                return h, last

            def qk(b, h_sb):
                """Q,K fp8 [d, hw] via DoubleRow fp8 matmuls; psum->sbuf
                copies on DVE."""
                q_sb = qkp.tile([P, NT, HW], f8, tag="q")
                k_sb = qkp.tile([P, NT, HW], f8, tag="k")
                # Q psum->sbuf copies on DVE, K copies on ACT: split across
                # engines so the psq pool drains at 2x copy rate and neither
                # engine's queue backs up.
                for dst_sb, w_sb, bias_idx, eng in (
                        (q_sb, wq_sb, 0, "dve"), (k_sb, wk_sb, 1, "act")):
                    for dm in range(NT):
                        ps0 = psq.tile([P, 512], f32, tag="qkv")
                        ps1 = psq.tile([P, 512], f32, tag="qkv")
                        for kp in range(NP):
                            lhsT = w_sb[:, 2 * kp:2 * kp + 2, dm * P:(dm + 1) * P]
                            st, sp = (kp == 0), (kp == NP - 1)
                            nc.tensor.matmul(ps0[:, :], lhsT,
                                             h_sb[:, 2 * kp:2 * kp + 2, 0:512],
                                             start=st, stop=sp, perf_mode=DR)
                            nc.tensor.matmul(ps1[:, :], lhsT,
                                             h_sb[:, 2 * kp:2 * kp + 2, 512:1024],
                                             start=st, stop=sp, perf_mode=DR)
                        for ps, nsl in ((ps0, slice(0, 512)), (ps1, slice(512, 1024))):
                            if zero_qk_bias:
                                if eng == "act":
                                    nc.scalar.copy(dst_sb[:, dm, nsl], ps[:, :])
                                else:
                                    nc.vector.tensor_copy(dst_sb[:, dm, nsl], ps[:, :])
                            else:
                                if eng == "act":
                                    nc.scalar.add(dst_sb[:, dm, nsl], ps[:, :],
                                                  bqkt_sb[:, bias_idx, dm:dm + 1])
                                else:
                                    nc.vector.tensor_scalar(
                                        out=dst_sb[:, dm, nsl], in0=ps[:, :],
                                        scalar1=bqkt_sb[:, bias_idx, dm:dm + 1],
                                        scalar2=None, op0=ALU.add)
                return q_sb, k_sb

            def vmm(b, h_sb):
                """V bf16 transposed [hw, d]; psum->sbuf copies on ACT."""
                vt_sb = vtp.tile([P, NJ, C], bf16, tag="vt")
                for jm in range(NJ):
                    ps = psq.tile([P, 512], f32, tag="qkv")
                    for kp in range(NP):
                        nc.tensor.matmul(
                            ps[:, :],
                            h_sb[:, 2 * kp:2 * kp + 2, jm * P:(jm + 1) * P],
                            wv_sb[:, 2 * kp:2 * kp + 2, :],
                            start=(kp == 0), stop=(kp == NP - 1), perf_mode=DR,
                        )
                    if zero_v_bias:
                        nc.scalar.copy(vt_sb[:, jm, :], ps[:, :])
                    else:
                        nc.vector.tensor_add(vt_sb[:, jm, :], ps[:, :], bv_sb[:, :])
                return vt_sb

            # ---------------- prologue: GN(0), qk(0), GN(1) ----------------
            # V(0) is NOT issued here: its ACT copies would queue ahead of
            # exp(0) and delay the whole pipeline. It is issued inside
            # iteration 0 (after the den section), mirroring the steady-state
            # vmm(b+1) position, so its copies run after exp(0) drains.
            mv0 = gn_stats(0)
            gn_affine(0, mv0)
            h0_sb, last_apply = apply_h(0)
            h_next = h0_sb

            qk_next = qk(0, h0_sb)

            mv1 = gn_stats(1, after=last_apply)
            gn_affine(1, mv1)
            h_next, last_apply = apply_h(1)

            vt_next = None

            # ---------------- main loop ----------------
            for b in range(NB):
                q_sb, k_sb = qk_next
                vt_sb = vt_next
                h_sb = h_next

                # ---- scores transposed + exp (ACT) ----
                est_sb = estp.tile([P, NJ, HW], bf16, tag="est")
                for jm in range(NJ):
                    st_ps = psb.tile([P, HW], f32, tag="big")
                    for kp in range(NP):
                        lhsT = k_sb[:, 2 * kp:2 * kp + 2, jm * P:(jm + 1) * P]
                        st, sp = (kp == 0), (kp == NP - 1)
                        nc.tensor.matmul(st_ps[:, 0:512], lhsT,
                                         q_sb[:, 2 * kp:2 * kp + 2, 0:512],
                                         start=st, stop=sp, perf_mode=DR)
                        nc.tensor.matmul(st_ps[:, 512:1024], lhsT,
                                         q_sb[:, 2 * kp:2 * kp + 2, 512:1024],
                                         start=st, stop=sp, perf_mode=DR)
                    nc.scalar.activation(
                        out=est_sb[:, jm, :], in_=st_ps[:, :], func=ACT.Exp,
                        scale=CINV,
                    )

                # ---- softmax denominator ----
                # pairwise bf16 tree over the 8 exp tiles (DVE 2x rate), then
                # one (1/16)-matrix bf16 matmul reduces the last 128
                # partitions AND replicates den/16 across partitions. Issued
                # ahead of qk(b+1) so recip/rep unblocks PV's O-scales early.
                s0 = sump.tile([P, HW], bf16, tag="s0")
                s1 = sump.tile([P, HW], bf16, tag="s1")
                s2 = sump.tile([P, HW], bf16, tag="s2")
                s3 = sump.tile([P, HW], bf16, tag="s3")
                nc.vector.tensor_add(s0[:, :], est_sb[:, 0, :], est_sb[:, 1, :])
                nc.vector.tensor_add(s1[:, :], est_sb[:, 2, :], est_sb[:, 3, :])
                nc.vector.tensor_add(s2[:, :], est_sb[:, 4, :], est_sb[:, 5, :])
                nc.vector.tensor_add(s3[:, :], est_sb[:, 6, :], est_sb[:, 7, :])
                nc.vector.tensor_add(s0[:, :], s0[:, :], s1[:, :])
                nc.vector.tensor_add(s2[:, :], s2[:, :], s3[:, :])
                nc.vector.tensor_add(s0[:, :], s0[:, :], s2[:, :])

                # sample 0's V, deferred out of the prologue (see above)
                if b == 0:
                    vt_sb = vmm(0, h0_sb)

                # ---- Q,K for sample b+1 fill TensorE while ACT exps ----
                if b + 1 < NB:
                    qk_next = qk(b + 1, h_sb)

                # den's partition-reduce matmuls AFTER the qk/vmm matmuls in
                # the TensorE stream: they block on the DVE tree (which blocks
                # on exp), and rep is only needed by PV's first O-scale.
                rs0 = psq.tile([P, 512], f32, tag="qkv")
                rs1 = psq.tile([P, 512], f32, tag="qkv")
                nc.tensor.matmul(rs0[:, :], ones_sb[:, :], s0[:, 0:512],
                                 start=True, stop=True)
                nc.tensor.matmul(rs1[:, :], ones_sb[:, :], s0[:, 512:1024],
                                 start=True, stop=True)
                rep = op.tile([P, HW], f32, tag="rep")
                nc.vector.reciprocal_approx_fast(out=rep[:, 0:512], in_=rs0[:, :])
                nc.vector.reciprocal_approx_fast(out=rep[:, 512:1024], in_=rs1[:, :])

                # ---- PV (bf16): O16[c, i] = (sum_j VT[j, c] * est[j, i]) * rep ----
                o_sb = op.tile([P, NT, HW], f8, tag="o")
                for cm in range(NT):
                    o_ps = psb.tile([P, HW], f32, tag="big")
                    for jm in range(NJ):
                        lhsT = vt_sb[:, jm, cm * P:(cm + 1) * P]
                        st, sp = (jm == 0), (jm == NJ - 1)
                        nc.tensor.matmul(o_ps[:, 0:512], lhsT,
                                         est_sb[:, jm, 0:512], start=st, stop=sp)
                        nc.tensor.matmul(o_ps[:, 512:1024], lhsT,
                                         est_sb[:, jm, 512:1024], start=st, stop=sp)
                    last_oscale = nc.vector.tensor_mul(
                        o_sb[:, cm, :], o_ps[:, :], rep[:, :])

                # ---- V for sample b+1 (between PV and proj: its ACT copies
                #      run right after exp(b) ends, before exp(b+1) needs ACT)
                if b + 1 < NB:
                    vt_next = vmm(b + 1, h_sb)

                # sample b+2's GN: stats are a low-priority DVE filler; the
                # affine's tiny PE matmul embeds in the stream; the DVE
                # applies are ordered behind this sample's O-scales.
                if b + 2 < NB:
                    mv_next = gn_stats(b + 2, after=last_apply)
                    gn_affine(b + 2, mv_next)
                    h_next, last_apply = apply_h(b + 2, after=last_oscale)

                # ---- out-proj (fp8 DR on 16x o) + 1/16 + residual ----
                for dm in range(NT):
                    p_ps = psb.tile([P, HW], f32, tag="big")
                    for kp in range(NP):
                        lhsT = wt_sb[:, 2 * kp:2 * kp + 2, dm * P:(dm + 1) * P]
                        st, sp = (kp == 0), (kp == NP - 1)
                        nc.tensor.matmul(p_ps[:, 0:512], lhsT,
                                         o_sb[:, 2 * kp:2 * kp + 2, 0:512],
                                         start=st, stop=sp, perf_mode=DR)
                        nc.tensor.matmul(p_ps[:, 512:1024], lhsT,
                                         o_sb[:, 2 * kp:2 * kp + 2, 512:1024],
                                         start=st, stop=sp, perf_mode=DR)
                    out_t = outp.tile([P, HW], f32, tag="out")
                    if b == NB - 1:
                        # quarter-split the final residuals so the last
                        # out-DMAs start as early as possible
                        qengs = (nc.sync, nc.gpsimd, nc.scalar, nc.sync)
                        for qq in range(4):
                            h0 = qq * 256
                            if zero_t_bias:
                                nc.vector.scalar_tensor_tensor(
                                    out=out_t[:, h0:h0 + 256],
                                    in0=p_ps[:, h0:h0 + 256],
                                    scalar=1.0 / OSC,
                                    in1=x_sb[:, b, dm, h0:h0 + 256],
                                    op0=ALU.mult, op1=ALU.add,
                                )
                            else:
                                nc.vector.tensor_scalar(
                                    out=out_t[:, h0:h0 + 256],
                                    in0=p_ps[:, h0:h0 + 256],
                                    scalar1=1.0 / OSC,
                                    scalar2=bqkt_sb[:, 2, dm:dm + 1],
                                    op0=ALU.mult, op1=ALU.add)
                                nc.vector.tensor_add(
                                    out_t[:, h0:h0 + 256], out_t[:, h0:h0 + 256],
                                    x_sb[:, b, dm, h0:h0 + 256])
                            qengs[qq].dma_start(
                                out=out_d[b, dm * P:(dm + 1) * P, h0:h0 + 256],
                                in_=out_t[:, h0:h0 + 256])
                    else:
                        if zero_t_bias:
                            nc.vector.scalar_tensor_tensor(
                                out=out_t[:, :], in0=p_ps[:, :],
                                scalar=1.0 / OSC, in1=x_sb[:, b, dm, :],
                                op0=ALU.mult, op1=ALU.add,
                            )
                        else:
                            nc.vector.tensor_scalar(
                                out=out_t[:, :], in0=p_ps[:, :],
                                scalar1=1.0 / OSC,
                                scalar2=bqkt_sb[:, 2, dm:dm + 1],
                                op0=ALU.mult, op1=ALU.add)
                            nc.vector.tensor_add(
                                out_t[:, :], out_t[:, :], x_sb[:, b, dm, :])
                        nc.sync.dma_start(
                            out=out_d[b, dm * P:(dm + 1) * P, 0:512],
                            in_=out_t[:, 0:512])
                        nc.gpsimd.dma_start(
                            out=out_d[b, dm * P:(dm + 1) * P, 512:1024],
                            in_=out_t[:, 512:1024])

    nc.compile()
    return nc


def prep_inputs(inputs):
    """Host-side prep: per-core in_maps with pre-laid-out weights/constants."""
    e4 = ml_dtypes.float8_e4m3
    x = np.ascontiguousarray(np.asarray(inputs["x"], dtype=np.float32)).reshape(
        B_FULL, C, HW
    )

    def wprep(w):
        # [C, C] -> [P, NT, C]  (lhsT slices w[kc*128+p, d])
        return np.ascontiguousarray(
            np.asarray(w, dtype=np.float32).reshape(NT, P, C).transpose(1, 0, 2)
        ).astype(e4)

    def cols(v):
        # [C] -> [P, NT]
        return np.ascontiguousarray(
            np.asarray(v, dtype=np.float32).reshape(NT, P).T
        )

    bqkt = np.stack([cols(inputs["bq"]), cols(inputs["bk"]), cols(inputs["bt"])],
                    axis=1)  # [P, 3, NT]
    gab = np.stack([cols(inputs["gn_scale"]), cols(inputs["gn_bias"])], axis=1)
    bv_rep = np.tile(np.asarray(inputs["bv"], dtype=np.float32)[None, :], (P, 1))
    gg = np.zeros((P, P), np.float32)
    for p in range(P):
        gg[p, (p // GS) * GS:(p // GS + 1) * GS] = 1.0 / GS

    shared = {
        "wq": wprep(inputs["Wq"]), "wk": wprep(inputs["Wk"]),
        "wv": wprep(inputs["Wv"]), "wt": wprep(inputs["Wt"]),
        "bqkt": np.ascontiguousarray(bqkt), "bv_rep": bv_rep,
        "gn_ab": np.ascontiguousarray(gab), "gg": gg,
    }
    in_maps = []
    for c_id in range(N_CORES):
        m = dict(shared)
        m["x"] = np.ascontiguousarray(x[c_id * NB:(c_id + 1) * NB])
        in_maps.append(m)
    return in_maps


_NC_CACHE = {}


def get_nc(zero_qk_bias=True, zero_v_bias=True, zero_t_bias=True):
    key = (zero_qk_bias, zero_v_bias, zero_t_bias)
    if key not in _NC_CACHE:
        _NC_CACHE[key] = build_nc(zero_qk_bias=zero_qk_bias,
                                  zero_v_bias=zero_v_bias,
                                  zero_t_bias=zero_t_bias)
    return _NC_CACHE[key]


def run(inputs, trace=False):
    from concourse.bass_utils import run_bass_kernel_spmd

    zq = bool(
        np.all(np.asarray(inputs["bq"]) == 0) and np.all(np.asarray(inputs["bk"]) == 0)
    )
    zv = bool(np.all(np.asarray(inputs["bv"]) == 0))
    zt = bool(np.all(np.asarray(inputs["bt"]) == 0))
    nc = get_nc(zero_qk_bias=zq, zero_v_bias=zv, zero_t_bias=zt)
    in_maps = prep_inputs(inputs)
    res = run_bass_kernel_spmd(
        nc, in_maps, core_ids=list(range(N_CORES)), trace=trace
    )
    out = np.concatenate([np.asarray(r["out"]) for r in res.results], axis=0)
    return out.reshape(B_FULL, C, H, W), res


def kernel(**inputs):
    out, _ = run(inputs, trace=False)
    return out


# revision 24
# speedup vs baseline: 1.0190x; 1.0108x over previous
"""AttnBlock (GroupNorm -> QKV 1x1 -> full attention over 1024 tokens -> out-proj
+ residual) for x [32, 512, 32, 32] f32, distributed data-parallel over 8
NeuronCores (4 samples per core, weights replicated).

Per-core single-NC Bass/Tile kernel. fp8(e4m3) DoubleRow TensorE compute for
the four 1x1 projections and the score matmul (2 contraction rows per PE
cell -> ~1.5x bf16 throughput); bf16 for the PV matmul (softmax-weight fp8
quantization dominates the error budget, so est stays bf16); f32 softmax
stats.

  - GroupNorm via per-channel bn_stats/bn_aggr + tiny selector matmuls for the
    cross-partition group reduce (fp32), fast-inverse-sqrt on DVE.
  - h, q, k stored fp8; scores computed TRANSPOSED (ST[j,i] = sum_d K[d,j]
    Q[d,i]) so P~ = exp(ST*c) has the contraction axis j on partitions and PV
    needs no transposes. V computed transposed ([hw, d], bf16 out).
  - Softmax denominator: DVE pairwise-add tree over the 8 exp tiles in bf16
    (2x DVE rate), then one (1/16)-matrix bf16 stationary matmul that reduces
    the remaining 128 partitions and replicates den/16 across partitions;
    rep = 16/den via reciprocal_approx_fast. The PV psum->sbuf copy multiplies
    by rep, so o is stored as fp8 at 16x scale (lands in e4m3 normal range);
    the final residual op multiplies the out-proj psum by 1/16.
  - Cross-sample software pipeline: TensorE order per iteration b is
    [scores(b) | QK(b+1) | PV(b) | V(b+1) | proj(b)] so the ACT exp stream
    (the slowest per-phase engine, ~1.1us/tile vs 0.75us/tile for the fp8
    score matmuls) of sample b is hidden behind the QK matmuls of sample b+1.
    GroupNorm for sample b+2 runs on DVE during iteration b. The psum->sbuf
    copies are split across engines (Q on DVE, K and V on ACT after the exp
    stream drains; GPSIMD cannot read PSUM) so the psq pool never backs up
    into the TensorE stream.
"""

import os
import sys

import numpy as np

sys.path.insert(0, "/opt/trn_rl_repo")

import ml_dtypes  # noqa: E402

import concourse.bass as bass  # noqa: E402
import concourse.tile as tile  # noqa: E402
from concourse import bacc, mybir  # noqa: E402

P = 128
B_FULL, C, H, W = 32, 512, 32, 32
HW = H * W            # 1024 tokens
N_CORES = 8
NB = B_FULL // N_CORES  # 4 samples per core
NT = C // P           # 4 channel tiles
NP = NT // 2          # 2 DoubleRow channel-tile pairs
NJ = HW // P          # 8 token tiles
NGROUPS = 32
GS = C // NGROUPS     # 16 channels per group
G_PER_TILE = P // GS  # 8 groups per 128-channel tile
EPS = 1e-6
CINV = float(C) ** -0.5
OSC = 16.0            # fp8 o is stored at 16x scale

f32 = mybir.dt.float32
bf16 = mybir.dt.bfloat16
f8 = mybir.dt.float8e4
ALU = mybir.AluOpType
ACT = mybir.ActivationFunctionType
DR = mybir.MatmulPerfMode.DoubleRow


def build_nc(zero_qk_bias=False, zero_v_bias=False, zero_t_bias=False):
    """Build the single-core Bass graph (SPMD: same graph on all 8 cores).

    zero_*_bias: when the corresponding biases are all-zero (true for this
    problem's setup_inputs), the psum->sbuf copies drop the bias add.
    """
    nc = bacc.Bacc("TRN2", target_bir_lowering=False, debug=False)

    x_d = nc.dram_tensor("x", [NB, C, HW], f32, kind="ExternalInput")
    wq_d = nc.dram_tensor("wq", [P, NT, C], f8, kind="ExternalInput")
    wk_d = nc.dram_tensor("wk", [P, NT, C], f8, kind="ExternalInput")
    wv_d = nc.dram_tensor("wv", [P, NT, C], f8, kind="ExternalInput")
    wt_d = nc.dram_tensor("wt", [P, NT, C], f8, kind="ExternalInput")
    # packed per-partition bias columns: [:, 0, :]=bq, [:, 1, :]=bk, [:, 2, :]=bt
    bqkt_d = nc.dram_tensor("bqkt", [P, 3, NT], f32, kind="ExternalInput")
    bv_d = nc.dram_tensor("bv_rep", [P, C], f32, kind="ExternalInput")
    # gn affine columns: [:, 0, :]=gamma, [:, 1, :]=beta
    gab_d = nc.dram_tensor("gn_ab", [P, 2, NT], f32, kind="ExternalInput")
    # block-diagonal group-average matrix: GG[k,p] = 1/16 iff k//16 == p//16
    gg_d = nc.dram_tensor("gg", [P, P], f32, kind="ExternalInput")
    out_d = nc.dram_tensor("out", [NB, C, HW], f32, kind="ExternalOutput")

    with tile.TileContext(nc) as tc:
        with (
            tc.tile_pool(name="consts", bufs=1) as consts,
            tc.tile_pool(name="hp", bufs=2) as hp,
            tc.tile_pool(name="qkp", bufs=1) as qkp,
            tc.tile_pool(name="vtp", bufs=2) as vtp,
            tc.tile_pool(name="est", bufs=1) as estp,
            tc.tile_pool(name="op", bufs=1) as op,
            tc.tile_pool(name="outp", bufs=3) as outp,
            tc.tile_pool(name="small", bufs=2) as small,
            tc.tile_pool(name="sump", bufs=1) as sump,
            tc.tile_pool(name="psb", bufs=2, space="PSUM") as psb,
            tc.tile_pool(name="psq", bufs=4, space="PSUM") as psq,
        ):
            # ---- x[0] gates everything: one tile per DMA queue (4 parallel
            #      rings), then weights/consts sequenced by first-use time.
            x_sb = consts.tile([P, NB, NT, HW], f32, tag="x")
            wq_sb = consts.tile([P, NT, C], f8, tag="wq")
            wk_sb = consts.tile([P, NT, C], f8, tag="wk")
            wv_sb = consts.tile([P, NT, C], f8, tag="wv")
            wt_sb = consts.tile([P, NT, C], f8, tag="wt")
            gab_sb = consts.tile([P, 2, NT], f32, tag="gab")
            gg_sb = consts.tile([P, P], f32, tag="gg")
            bqkt_sb = consts.tile([P, 3, NT], f32, tag="bqkt")
            bv_sb = consts.tile([P, C], f32, tag="bv")

            # x[0] quarters round-robin across the 3 DMA queues (x[0] wire
            # time gates the GN(0) -> QK(0) chain); gg+gab first on gpsimd
            # (tiny, needed by affine(0)).
            nc.gpsimd.dma_start(out=gg_sb[:, :], in_=gg_d[:, :])
            nc.gpsimd.dma_start(out=gab_sb[:, :, :], in_=gab_d[:, :, :])
            engs = (nc.sync, nc.scalar, nc.gpsimd)
            qi = 0
            for t in range(NT):
                for h0 in (0, 256, 512, 768):
                    engs[qi % 3].dma_start(
                        out=x_sb[:, 0, t, h0:h0 + 256],
                        in_=x_d[0, t * P:(t + 1) * P, h0:h0 + 256])
                    qi += 1
            # weights right behind x[0] (land ~when QK(0)/V(0) start)
            nc.sync.dma_start(out=wq_sb[:, :, :], in_=wq_d[:, :, :])
            nc.scalar.dma_start(out=wk_sb[:, :, :], in_=wk_d[:, :, :])
            nc.gpsimd.dma_start(out=wv_sb[:, :, :], in_=wv_d[:, :, :])
            # x[1] split sync/scalar (needed by stats(1) mid-prologue)
            for t in (0, 1):
                nc.scalar.dma_start(out=x_sb[:, 1, t, :],
                                    in_=x_d[1, t * P:(t + 1) * P, :])
            for t in (2, 3):
                nc.sync.dma_start(out=x_sb[:, 1, t, :],
                                  in_=x_d[1, t * P:(t + 1) * P, :])
            # x[3] on sync (needed iteration 1)
            for t in range(NT):
                nc.sync.dma_start(out=x_sb[:, 3, t, :],
                                  in_=x_d[3, t * P:(t + 1) * P, :])
            # gpsimd: remaining consts + x[2] + wt
            nc.gpsimd.dma_start(out=bqkt_sb[:, :, :], in_=bqkt_d[:, :, :])
            for t in range(NT):
                nc.gpsimd.dma_start(out=x_sb[:, 2, t, :],
                                    in_=x_d[2, t * P:(t + 1) * P, :])
            nc.gpsimd.dma_start(out=bv_sb[:, :], in_=bv_d[:, :])
            nc.gpsimd.dma_start(out=wt_sb[:, :, :], in_=wt_d[:, :, :])
            # (1/16)-matrix: reduces partitions AND folds den/16 for the
            # 16x fp8 o-scale
            ones_sb = consts.tile([P, P], bf16, tag="ones")
            nc.vector.memset(ones_sb[:, :], 1.0 / OSC)
            magic_sb = consts.tile([P, NT], mybir.dt.int32, tag="magic")
            nc.vector.memset(magic_sb[:, :], 0x5F3759DF)
            # dummy Exp: pulls the ACT Exp-table load into the idle prologue
            # (off the first real exp's critical path)
            expwarm = small.tile([P, 1], f32, tag="expwarm")
            nc.scalar.activation(out=expwarm[:, :], in_=ones_sb[:, 0:1],
                                 func=ACT.Exp)

            a_all = consts.tile([P, NB, NT], f32, tag="a_all")
            b_all = consts.tile([P, NB, NT], f32, tag="b_all")

            # PE warm-up: harmless fp32 matmuls on the earliest-arriving x
            # tile so the HAM clock-gate is released before the real stream.
            warm_ps = psq.tile([P, 512], f32, tag="qkv")
            for w in range(5):
                nc.tensor.matmul(
                    warm_ps[:, :], x_sb[:, 0, 0, 0:128], x_sb[:, 0, 0, 0:512],
                    start=(w == 0), stop=(w == 4),
                )

            def gn_stats(b, after=None):
                """bn stats -> per-channel (mean, Ex2) packed in mv."""
                mv = small.tile([P, NT, 2], f32, tag="mv")
                nsub = 2
                step = HW // nsub
                for t in range(NT):
                    st6 = small.tile([P, nsub, 6], f32, tag="st6")
                    for q in range(nsub):
                        iq = nc.vector.bn_stats(
                            out=st6[:, q, :],
                            in_=x_sb[:, b, t, q * step:(q + 1) * step])
                        if after is not None:
                            tile.add_dep_helper(iq.ins, after.ins, sync=False,
                                                reason="gn stats after prev apply")
                    nc.vector.bn_aggr(out=mv[:, t, :], in_=st6[:, :, :])
                msq = small.tile([P, NT], f32, tag="msq")
                nc.vector.tensor_mul(msq[:, :], mv[:, :, 0], mv[:, :, 0])
                nc.vector.tensor_add(mv[:, :, 1], mv[:, :, 1], msq[:, :])
                return mv

            def gn_affine(b, mv, use_act_sqrt=False):
                """fused group-avg+broadcast matmul, then form per-channel A/B."""
                bc_ps = psq.tile([P, 512], f32, tag="qkv")
                nc.tensor.matmul(bc_ps[:, :NT * 2], gg_sb[:, :], mv[:, :, :],
                                 start=True, stop=True)
                bc = small.tile([P, NT, 2], f32, tag="bcs")
                nc.vector.tensor_copy(bc[:, :, :], bc_ps[:, 0:NT * 2])
                vb = small.tile([P, NT], f32, tag="vb")
                nc.vector.tensor_mul(vb[:, :], bc[:, :, 0], bc[:, :, 0])
                nc.vector.tensor_sub(vb[:, :], bc[:, :, 1], vb[:, :])
                if use_act_sqrt:
                    # sample 0 (pre-exp): the shorter ACT chain wins and its
                    # Sqrt table load cannot evict a not-yet-loaded Exp table
                    nc.vector.tensor_scalar_add(vb[:, :], vb[:, :], EPS)
                    nc.scalar.sqrt(vb[:, :], vb[:, :])
                    nc.vector.reciprocal(vb[:, :], vb[:, :])
                    y1 = vb
                else:
                    nc.vector.tensor_scalar_add(vb[:, :], vb[:, :], EPS)
                    # rstd = rsqrt(var+eps): fast-inverse-sqrt + 2 Newton steps
                    # (all-DVE: keeps Sqrt off ACT so it never evicts Exp)
                    ii = small.tile([P, NT], mybir.dt.int32, tag="ii")
                    nc.vector.tensor_scalar(
                        out=ii[:, :], in0=vb.bitcast(mybir.dt.int32)[:, :],
                        scalar1=1, scalar2=None, op0=ALU.arith_shift_right)
                    nc.vector.tensor_tensor(ii[:, :], magic_sb[:, :], ii[:, :],
                                            op=ALU.subtract)
                    y0 = ii.bitcast(f32)
                    yt = small.tile([P, NT], f32, tag="yt")
                    y1 = small.tile([P, NT], f32, tag="y1")
                    nc.vector.tensor_mul(yt[:, :], vb[:, :], y0[:, :])
                    nc.vector.tensor_mul(yt[:, :], yt[:, :], y0[:, :])
                    nc.vector.tensor_scalar(out=yt[:, :], in0=yt[:, :], scalar1=-0.5,
                                            scalar2=1.5, op0=ALU.mult, op1=ALU.add)
                    nc.vector.tensor_mul(y1[:, :], y0[:, :], yt[:, :])
                    nc.vector.tensor_mul(yt[:, :], vb[:, :], y1[:, :])
                    nc.vector.tensor_mul(yt[:, :], yt[:, :], y1[:, :])
                    nc.vector.tensor_scalar(out=yt[:, :], in0=yt[:, :], scalar1=-0.5,
                                            scalar2=1.5, op0=ALU.mult, op1=ALU.add)
                    nc.vector.tensor_mul(y1[:, :], y1[:, :], yt[:, :])
                tmp = small.tile([P, NT], f32, tag="tmpab")
                nc.vector.tensor_mul(a_all[:, b, :], y1[:, :], gab_sb[:, 0, :])
                nc.vector.tensor_mul(tmp[:, :], bc[:, :, 0], a_all[:, b, :])
                nc.vector.tensor_sub(b_all[:, b, :], gab_sb[:, 1, :], tmp[:, :])

            def apply_h(b, after=None):
                """h = x*A + B (fp8)"""
                h = hp.tile([P, NT, HW], f8, tag="h")
                last = None
                for t in range(NT):
                    last = nc.vector.tensor_scalar(
                        out=h[:, t, :], in0=x_sb[:, b, t, :],
                        scalar1=a_all[:, b, t:t + 1], scalar2=b_all[:, b, t:t + 1],
                        op0=ALU.mult, op1=ALU.add,
                    )
                    if after is not None:
                        tile.add_dep_helper(last.ins, after.ins, sync=False,
                                            reason="apply after O-scales")
                return h, last

            def qk(b, h_sb):
                """Q,K fp8 [d, hw] via DoubleRow fp8 matmuls; psum->sbuf
                copies on DVE."""
                q_sb = qkp.tile([P, NT, HW], f8, tag="q")
                k_sb = qkp.tile([P, NT, HW], f8, tag="k")
                # Q psum->sbuf copies on DVE, K copies on ACT: split across
                # engines so the psq pool drains at 2x copy rate and neither
                # engine's queue backs up.
                for dst_sb, w_sb, bias_idx, eng in (
                        (q_sb, wq_sb, 0, "dve"), (k_sb, wk_sb, 1, "act")):
                    for dm in range(NT):
                        ps0 = psq.tile([P, 512], f32, tag="qkv")
                        ps1 = psq.tile([P, 512], f32, tag="qkv")
                        for kp in range(NP):
                            lhsT = w_sb[:, 2 * kp:2 * kp + 2, dm * P:(dm + 1) * P]
                            st, sp = (kp == 0), (kp == NP - 1)
                            nc.tensor.matmul(ps0[:, :], lhsT,
                                             h_sb[:, 2 * kp:2 * kp + 2, 0:512],
                                             start=st, stop=sp, perf_mode=DR)
                            nc.tensor.matmul(ps1[:, :], lhsT,
                                             h_sb[:, 2 * kp:2 * kp + 2, 512:1024],
                                             start=st, stop=sp, perf_mode=DR)
                        for ps, nsl in ((ps0, slice(0, 512)), (ps1, slice(512, 1024))):
                            if zero_qk_bias:
                                if eng == "act":
                                    nc.scalar.copy(dst_sb[:, dm, nsl], ps[:, :])
                                else:
                                    nc.vector.tensor_copy(dst_sb[:, dm, nsl], ps[:, :])
                            else:
                                if eng == "act":
                                    nc.scalar.add(dst_sb[:, dm, nsl], ps[:, :],
                                                  bqkt_sb[:, bias_idx, dm:dm + 1])
                                else:
                                    nc.vector.tensor_scalar(
                                        out=dst_sb[:, dm, nsl], in0=ps[:, :],
                                        scalar1=bqkt_sb[:, bias_idx, dm:dm + 1],
                                        scalar2=None, op0=ALU.add)
                return q_sb, k_sb

            def vmm(b, h_sb):
                """V bf16 transposed [hw, d]; psum->sbuf copies on ACT."""
                vt_sb = vtp.tile([P, NJ, C], bf16, tag="vt")
                for jm in range(NJ):
                    ps = psq.tile([P, 512], f32, tag="qkv")
                    for kp in range(NP):
                        nc.tensor.matmul(
                            ps[:, :],
                            h_sb[:, 2 * kp:2 * kp + 2, jm * P:(jm + 1) * P],
                            wv_sb[:, 2 * kp:2 * kp + 2, :],
                            start=(kp == 0), stop=(kp == NP - 1), perf_mode=DR,
                        )
                    if zero_v_bias:
                        nc.scalar.copy(vt_sb[:, jm, :], ps[:, :])
                    else:
                        nc.vector.tensor_add(vt_sb[:, jm, :], ps[:, :], bv_sb[:, :])
                return vt_sb

            # ---------------- prologue: GN(0), qk(0), GN(1) ----------------
            # V(0) is NOT issued here: its ACT copies would queue ahead of
            # exp(0) and delay the whole pipeline. It is issued inside
            # iteration 0 (after the den section), mirroring the steady-state
            # vmm(b+1) position, so its copies run after exp(0) drains.
            mv0 = gn_stats(0)
            gn_affine(0, mv0)
            h0_sb, last_apply = apply_h(0)
            h_next = h0_sb

            qk_next = qk(0, h0_sb)

            mv1 = gn_stats(1, after=last_apply)
            gn_affine(1, mv1)
            h_next, last_apply = apply_h(1)

            vt_next = None

            # ---------------- main loop ----------------
            for b in range(NB):
                q_sb, k_sb = qk_next
                vt_sb = vt_next
                h_sb = h_next

                # ---- scores transposed + exp (ACT) ----
                est_sb = estp.tile([P, NJ, HW], bf16, tag="est")
                for jm in range(NJ):
                    st_ps = psb.tile([P, HW], f32, tag="big")
                    for kp in range(NP):
                        lhsT = k_sb[:, 2 * kp:2 * kp + 2, jm * P:(jm + 1) * P]
                        st, sp = (kp == 0), (kp == NP - 1)
                        nc.tensor.matmul(st_ps[:, 0:512], lhsT,
                                         q_sb[:, 2 * kp:2 * kp + 2, 0:512],
                                         start=st, stop=sp, perf_mode=DR)
                        nc.tensor.matmul(st_ps[:, 512:1024], lhsT,
                                         q_sb[:, 2 * kp:2 * kp + 2, 512:1024],
                                         start=st, stop=sp, perf_mode=DR)
                    nc.scalar.activation(
                        out=est_sb[:, jm, :], in_=st_ps[:, :], func=ACT.Exp,
                        scale=CINV,
                    )

                # ---- softmax denominator ----
                # pairwise bf16 tree over the 8 exp tiles (DVE 2x rate), then
                # one (1/16)-matrix bf16 matmul reduces the last 128
                # partitions AND replicates den/16 across partitions. Issued
                # ahead of qk(b+1) so recip/rep unblocks PV's O-scales early.
                s0 = sump.tile([P, HW], bf16, tag="s0")
                s1 = sump.tile([P, HW], bf16, tag="s1")
                s2 = sump.tile([P, HW], bf16, tag="s2")
                s3 = sump.tile([P, HW], bf16, tag="s3")
                nc.vector.tensor_add(s0[:, :], est_sb[:, 0, :], est_sb[:, 1, :])
                nc.vector.tensor_add(s1[:, :], est_sb[:, 2, :], est_sb[:, 3, :])
                nc.vector.tensor_add(s2[:, :], est_sb[:, 4, :], est_sb[:, 5, :])
                nc.vector.tensor_add(s3[:, :], est_sb[:, 6, :], est_sb[:, 7, :])
                nc.vector.tensor_add(s0[:, :], s0[:, :], s1[:, :])
                nc.vector.tensor_add(s2[:, :], s2[:, :], s3[:, :])
                nc.vector.tensor_add(s0[:, :], s0[:, :], s2[:, :])

                # sample 0's V, deferred out of the prologue (see above)
                if b == 0:
                    vt_sb = vmm(0, h0_sb)

                # ---- Q,K for sample b+1 fill TensorE while ACT exps ----
                if b + 1 < NB:
                    qk_next = qk(b + 1, h_sb)

                # den's partition-reduce matmuls AFTER the qk/vmm matmuls in
                # the TensorE stream: they block on the DVE tree (which blocks
                # on exp), and rep is only needed by PV's first O-scale.
                rs0 = psq.tile([P, 512], f32, tag="qkv")
                rs1 = psq.tile([P, 512], f32, tag="qkv")
                nc.tensor.matmul(rs0[:, :], ones_sb[:, :], s0[:, 0:512],
                                 start=True, stop=True)
                nc.tensor.matmul(rs1[:, :], ones_sb[:, :], s0[:, 512:1024],
                                 start=True, stop=True)
                rep = op.tile([P, HW], f32, tag="rep")
                nc.vector.reciprocal_approx_fast(out=rep[:, 0:512], in_=rs0[:, :])
                nc.vector.reciprocal_approx_fast(out=rep[:, 512:1024], in_=rs1[:, :])

                # ---- PV (bf16): O16[c, i] = (sum_j VT[j, c] * est[j, i]) * rep ----
                o_sb = op.tile([P, NT, HW], f8, tag="o")
                for cm in range(NT):
                    o_ps = psb.tile([P, HW], f32, tag="big")
                    for jm in range(NJ):
                        lhsT = vt_sb[:, jm, cm * P:(cm + 1) * P]
                        st, sp = (jm == 0), (jm == NJ - 1)
                        nc.tensor.matmul(o_ps[:, 0:512], lhsT,
                                         est_sb[:, jm, 0:512], start=st, stop=sp)
                        nc.tensor.matmul(o_ps[:, 512:1024], lhsT,
                                         est_sb[:, jm, 512:1024], start=st, stop=sp)
                    last_oscale = nc.vector.tensor_mul(
                        o_sb[:, cm, :], o_ps[:, :], rep[:, :])

                # ---- V for sample b+1 (between PV and proj: its ACT copies
                #      run right after exp(b) ends, before exp(b+1) needs ACT)
                if b + 1 < NB:
                    vt_next = vmm(b + 1, h_sb)

                # ---- out-proj (fp8 DR on 16x o) + 1/16 + residual ----
                for dm in range(NT):
                    p_ps = psb.tile([P, HW], f32, tag="big")
                    for kp in range(NP):
                        lhsT = wt_sb[:, 2 * kp:2 * kp + 2, dm * P:(dm + 1) * P]
                        st, sp = (kp == 0), (kp == NP - 1)
                        nc.tensor.matmul(p_ps[:, 0:512], lhsT,
                                         o_sb[:, 2 * kp:2 * kp + 2, 0:512],
                                         start=st, stop=sp, perf_mode=DR)
                        nc.tensor.matmul(p_ps[:, 512:1024], lhsT,
                                         o_sb[:, 2 * kp:2 * kp + 2, 512:1024],
                                         start=st, stop=sp, perf_mode=DR)
                    out_t = outp.tile([P, HW], f32, tag="out")
                    if b == NB - 1:
                        # quarter-split the final residuals so the last
                        # out-DMAs start as early as possible
                        qengs = (nc.sync, nc.gpsimd, nc.scalar, nc.sync)
                        for qq in range(4):
                            h0 = qq * 256
                            if zero_t_bias:
                                nc.vector.scalar_tensor_tensor(
                                    out=out_t[:, h0:h0 + 256],
                                    in0=p_ps[:, h0:h0 + 256],
                                    scalar=1.0 / OSC,
                                    in1=x_sb[:, b, dm, h0:h0 + 256],
                                    op0=ALU.mult, op1=ALU.add,
                                )
                            else:
                                nc.vector.tensor_scalar(
                                    out=out_t[:, h0:h0 + 256],
                                    in0=p_ps[:, h0:h0 + 256],
                                    scalar1=1.0 / OSC,
                                    scalar2=bqkt_sb[:, 2, dm:dm + 1],
                                    op0=ALU.mult, op1=ALU.add)
                                nc.vector.tensor_add(
                                    out_t[:, h0:h0 + 256], out_t[:, h0:h0 + 256],
                                    x_sb[:, b, dm, h0:h0 + 256])
                            qengs[qq].dma_start(
                                out=out_d[b, dm * P:(dm + 1) * P, h0:h0 + 256],
                                in_=out_t[:, h0:h0 + 256])
                    else:
                        if zero_t_bias:
                            nc.vector.scalar_tensor_tensor(
                                out=out_t[:, :], in0=p_ps[:, :],
                                scalar=1.0 / OSC, in1=x_sb[:, b, dm, :],
                                op0=ALU.mult, op1=ALU.add,
                            )
                        else:
                            nc.vector.tensor_scalar(
                                out=out_t[:, :], in0=p_ps[:, :],
                                scalar1=1.0 / OSC,
                                scalar2=bqkt_sb[:, 2, dm:dm + 1],
                                op0=ALU.mult, op1=ALU.add)
                            nc.vector.tensor_add(
                                out_t[:, :], out_t[:, :], x_sb[:, b, dm, :])
                        nc.sync.dma_start(
                            out=out_d[b, dm * P:(dm + 1) * P, 0:512],
                            in_=out_t[:, 0:512])
                        nc.gpsimd.dma_start(
                            out=out_d[b, dm * P:(dm + 1) * P, 512:1024],
                            in_=out_t[:, 512:1024])

                # sample b+2's GN, issued AFTER proj so the residuals (which
                # free the psb psum bufs that scores(b+1) needs) keep DVE
                # priority; the GN chain then fills early-scores(b+1) DVE
                # idle. apply is still ordered behind this sample's O-scales.
                if b + 2 < NB:
                    mv_next = gn_stats(b + 2, after=last_apply)
                    gn_affine(b + 2, mv_next)
                    h_next, last_apply = apply_h(b + 2, after=last_oscale)

    nc.compile()
    return nc


def prep_inputs(inputs):
    """Host-side prep: per-core in_maps with pre-laid-out weights/constants."""
    e4 = ml_dtypes.float8_e4m3
    x = np.ascontiguousarray(np.asarray(inputs["x"], dtype=np.float32)).reshape(
        B_FULL, C, HW
    )

    def wprep(w):
        # [C, C] -> [P, NT, C]  (lhsT slices w[kc*128+p, d])
        return np.ascontiguousarray(
            np.asarray(w, dtype=np.float32).reshape(NT, P, C).transpose(1, 0, 2)
        ).astype(e4)

    def cols(v):
        # [C] -> [P, NT]
        return np.ascontiguousarray(
            np.asarray(v, dtype=np.float32).reshape(NT, P).T
        )

    bqkt = np.stack([cols(inputs["bq"]), cols(inputs["bk"]), cols(inputs["bt"])],
                    axis=1)  # [P, 3, NT]
    gab = np.stack([cols(inputs["gn_scale"]), cols(inputs["gn_bias"])], axis=1)
    bv_rep = np.tile(np.asarray(inputs["bv"], dtype=np.float32)[None, :], (P, 1))
    gg = np.zeros((P, P), np.float32)
    for p in range(P):
        gg[p, (p // GS) * GS:(p // GS + 1) * GS] = 1.0 / GS

    shared = {
        "wq": wprep(inputs["Wq"]), "wk": wprep(inputs["Wk"]),
        "wv": wprep(inputs["Wv"]), "wt": wprep(inputs["Wt"]),
        "bqkt": np.ascontiguousarray(bqkt), "bv_rep": bv_rep,
        "gn_ab": np.ascontiguousarray(gab), "gg": gg,
    }
    in_maps = []
    for c_id in range(N_CORES):
        m = dict(shared)
        m["x"] = np.ascontiguousarray(x[c_id * NB:(c_id + 1) * NB])
        in_maps.append(m)
    return in_maps


_NC_CACHE = {}


def get_nc(zero_qk_bias=True, zero_v_bias=True, zero_t_bias=True):
    key = (zero_qk_bias, zero_v_bias, zero_t_bias)
    if key not in _NC_CACHE:
        _NC_CACHE[key] = build_nc(zero_qk_bias=zero_qk_bias,
                                  zero_v_bias=zero_v_bias,
                                  zero_t_bias=zero_t_bias)
    return _NC_CACHE[key]


def run(inputs, trace=False):
    from concourse.bass_utils import run_bass_kernel_spmd

    zq = bool(
        np.all(np.asarray(inputs["bq"]) == 0) and np.all(np.asarray(inputs["bk"]) == 0)
    )
    zv = bool(np.all(np.asarray(inputs["bv"]) == 0))
    zt = bool(np.all(np.asarray(inputs["bt"]) == 0))
    nc = get_nc(zero_qk_bias=zq, zero_v_bias=zv, zero_t_bias=zt)
    in_maps = prep_inputs(inputs)
    res = run_bass_kernel_spmd(
        nc, in_maps, core_ids=list(range(N_CORES)), trace=trace
    )
    out = np.concatenate([np.asarray(r["out"]) for r in res.results], axis=0)
    return out.reshape(B_FULL, C, H, W), res


def kernel(**inputs):
    out, _ = run(inputs, trace=False)
    return out


# revision 26
# speedup vs baseline: 1.0373x; 1.0180x over previous
"""AttnBlock (GroupNorm -> QKV 1x1 -> full attention over 1024 tokens -> out-proj
+ residual) for x [32, 512, 32, 32] f32, distributed data-parallel over 8
NeuronCores (4 samples per core, weights replicated).

Per-core single-NC Bass/Tile kernel. fp8(e4m3) DoubleRow TensorE compute for
the four 1x1 projections and the score matmul (2 contraction rows per PE
cell -> ~1.5x bf16 throughput); bf16 for the PV matmul (softmax-weight fp8
quantization dominates the error budget, so est stays bf16); f32 softmax
stats.

  - GroupNorm via per-channel bn_stats/bn_aggr + tiny selector matmuls for the
    cross-partition group reduce (fp32), fast-inverse-sqrt on DVE.
  - h, q, k stored fp8; scores computed TRANSPOSED (ST[j,i] = sum_d K[d,j]
    Q[d,i]) so P~ = exp(ST*c) has the contraction axis j on partitions and PV
    needs no transposes. V computed transposed ([hw, d], bf16 out).
  - Softmax denominator: DVE pairwise-add tree over the 8 exp tiles in bf16
    (2x DVE rate), then one (1/16)-matrix bf16 stationary matmul that reduces
    the remaining 128 partitions and replicates den/16 across partitions;
    rep = 16/den via reciprocal_approx_fast. The PV psum->sbuf copy multiplies
    by rep, so o is stored as fp8 at 16x scale (lands in e4m3 normal range);
    the final residual op multiplies the out-proj psum by 1/16.
  - Cross-sample software pipeline: TensorE order per iteration b is
    [scores(b) | QK(b+1) | PV(b) | V(b+1) | proj(b)] so the ACT exp stream
    (the slowest per-phase engine, ~1.1us/tile vs 0.75us/tile for the fp8
    score matmuls) of sample b is hidden behind the QK matmuls of sample b+1.
    GroupNorm for sample b+2 runs on DVE during iteration b. The psum->sbuf
    copies are split across engines (Q on DVE, K and V on ACT after the exp
    stream drains; GPSIMD cannot read PSUM) so the psq pool never backs up
    into the TensorE stream.
"""

import os
import sys

import numpy as np

sys.path.insert(0, "/opt/trn_rl_repo")

import ml_dtypes  # noqa: E402

import concourse.bass as bass  # noqa: E402
import concourse.tile as tile  # noqa: E402
from concourse import bacc, mybir  # noqa: E402

P = 128
B_FULL, C, H, W = 32, 512, 32, 32
HW = H * W            # 1024 tokens
N_CORES = 8
NB = B_FULL // N_CORES  # 4 samples per core
NT = C // P           # 4 channel tiles
NP = NT // 2          # 2 DoubleRow channel-tile pairs
NJ = HW // P          # 8 token tiles
NGROUPS = 32
GS = C // NGROUPS     # 16 channels per group
G_PER_TILE = P // GS  # 8 groups per 128-channel tile
EPS = 1e-6
CINV = float(C) ** -0.5
OSC = 16.0            # fp8 o is stored at 16x scale

f32 = mybir.dt.float32
bf16 = mybir.dt.bfloat16
f8 = mybir.dt.float8e4
ALU = mybir.AluOpType
ACT = mybir.ActivationFunctionType
DR = mybir.MatmulPerfMode.DoubleRow


def build_nc(zero_qk_bias=False, zero_v_bias=False, zero_t_bias=False):
    """Build the single-core Bass graph (SPMD: same graph on all 8 cores).

    zero_*_bias: when the corresponding biases are all-zero (true for this
    problem's setup_inputs), the psum->sbuf copies drop the bias add.
    """
    nc = bacc.Bacc("TRN2", target_bir_lowering=False, debug=False)

    x_d = nc.dram_tensor("x", [NB, C, HW], f32, kind="ExternalInput")
    wq_d = nc.dram_tensor("wq", [P, NT, C], f8, kind="ExternalInput")
    wk_d = nc.dram_tensor("wk", [P, NT, C], f8, kind="ExternalInput")
    wv_d = nc.dram_tensor("wv", [P, NT, C], f8, kind="ExternalInput")
    wt_d = nc.dram_tensor("wt", [P, NT, C], f8, kind="ExternalInput")
    # packed per-partition bias columns: [:, 0, :]=bq, [:, 1, :]=bk, [:, 2, :]=bt
    bqkt_d = nc.dram_tensor("bqkt", [P, 3, NT], f32, kind="ExternalInput")
    bv_d = nc.dram_tensor("bv_rep", [P, C], f32, kind="ExternalInput")
    # gn affine columns: [:, 0, :]=gamma, [:, 1, :]=beta
    gab_d = nc.dram_tensor("gn_ab", [P, 2, NT], f32, kind="ExternalInput")
    # block-diagonal group-average matrix: GG[k,p] = 1/16 iff k//16 == p//16
    gg_d = nc.dram_tensor("gg", [P, P], f32, kind="ExternalInput")
    out_d = nc.dram_tensor("out", [NB, C, HW], f32, kind="ExternalOutput")

    with tile.TileContext(nc) as tc:
        with (
            tc.tile_pool(name="consts", bufs=1) as consts,
            tc.tile_pool(name="hp", bufs=2) as hp,
            tc.tile_pool(name="qkp", bufs=1) as qkp,
            tc.tile_pool(name="vtp", bufs=2) as vtp,
            tc.tile_pool(name="est", bufs=1) as estp,
            tc.tile_pool(name="op", bufs=1) as op,
            tc.tile_pool(name="outp", bufs=4) as outp,
            tc.tile_pool(name="small", bufs=2) as small,
            tc.tile_pool(name="sump", bufs=1) as sump,
            tc.tile_pool(name="psb", bufs=2, space="PSUM") as psb,
            tc.tile_pool(name="psq", bufs=4, space="PSUM") as psq,
        ):
            # ---- x[0] gates everything: one tile per DMA queue (4 parallel
            #      rings), then weights/consts sequenced by first-use time.
            x_sb = consts.tile([P, NB, NT, HW], f32, tag="x")
            wq_sb = consts.tile([P, NT, C], f8, tag="wq")
            wk_sb = consts.tile([P, NT, C], f8, tag="wk")
            wv_sb = consts.tile([P, NT, C], f8, tag="wv")
            wt_sb = consts.tile([P, NT, C], f8, tag="wt")
            gab_sb = consts.tile([P, 2, NT], f32, tag="gab")
            gg_sb = consts.tile([P, P], f32, tag="gg")
            bqkt_sb = consts.tile([P, 3, NT], f32, tag="bqkt")
            bv_sb = consts.tile([P, C], f32, tag="bv")

            # x[0] quarters round-robin across the 3 DMA queues (x[0] wire
            # time gates the GN(0) -> QK(0) chain); gg+gab first on gpsimd
            # (tiny, needed by affine(0)).
            nc.gpsimd.dma_start(out=gg_sb[:, :], in_=gg_d[:, :])
            nc.gpsimd.dma_start(out=gab_sb[:, :, :], in_=gab_d[:, :, :])
            engs = (nc.sync, nc.scalar, nc.gpsimd)
            qi = 0
            for t in range(NT):
                for h0 in (0, 256, 512, 768):
                    engs[qi % 3].dma_start(
                        out=x_sb[:, 0, t, h0:h0 + 256],
                        in_=x_d[0, t * P:(t + 1) * P, h0:h0 + 256])
                    qi += 1
            # weights right behind x[0] (land ~when QK(0)/V(0) start)
            nc.sync.dma_start(out=wq_sb[:, :, :], in_=wq_d[:, :, :])
            nc.scalar.dma_start(out=wk_sb[:, :, :], in_=wk_d[:, :, :])
            nc.gpsimd.dma_start(out=wv_sb[:, :, :], in_=wv_d[:, :, :])
            # x[1] split sync/scalar (needed by stats(1) mid-prologue)
            for t in (0, 1):
                nc.scalar.dma_start(out=x_sb[:, 1, t, :],
                                    in_=x_d[1, t * P:(t + 1) * P, :])
            for t in (2, 3):
                nc.sync.dma_start(out=x_sb[:, 1, t, :],
                                  in_=x_d[1, t * P:(t + 1) * P, :])
            # x[3] on sync (needed iteration 1)
            for t in range(NT):
                nc.sync.dma_start(out=x_sb[:, 3, t, :],
                                  in_=x_d[3, t * P:(t + 1) * P, :])
            # gpsimd: remaining consts + x[2] + wt
            nc.gpsimd.dma_start(out=bqkt_sb[:, :, :], in_=bqkt_d[:, :, :])
            for t in range(NT):
                nc.gpsimd.dma_start(out=x_sb[:, 2, t, :],
                                    in_=x_d[2, t * P:(t + 1) * P, :])
            nc.gpsimd.dma_start(out=bv_sb[:, :], in_=bv_d[:, :])
            nc.gpsimd.dma_start(out=wt_sb[:, :, :], in_=wt_d[:, :, :])
            # (1/16)-matrix: reduces partitions AND folds den/16 for the
            # 16x fp8 o-scale
            ones_sb = consts.tile([P, P], bf16, tag="ones")
            nc.vector.memset(ones_sb[:, :], 1.0 / OSC)
            magic_sb = consts.tile([P, NT], mybir.dt.int32, tag="magic")
            nc.vector.memset(magic_sb[:, :], 0x5F3759DF)
            # dummy Exp: pulls the ACT Exp-table load into the idle prologue
            # (off the first real exp's critical path)
            expwarm = small.tile([P, 1], f32, tag="expwarm")
            nc.scalar.activation(out=expwarm[:, :], in_=ones_sb[:, 0:1],
                                 func=ACT.Exp)

            a_all = consts.tile([P, NB, NT], f32, tag="a_all")
            b_all = consts.tile([P, NB, NT], f32, tag="b_all")

            # PE warm-up: harmless fp32 matmuls on the earliest-arriving x
            # tile so the HAM clock-gate is released before the real stream.
            warm_ps = psq.tile([P, 512], f32, tag="qkv")
            for w in range(5):
                nc.tensor.matmul(
                    warm_ps[:, :], x_sb[:, 0, 0, 0:128], x_sb[:, 0, 0, 0:512],
                    start=(w == 0), stop=(w == 4),
                )

            def gn_stats(b, after=None):
                """bn stats -> per-channel (mean, Ex2) packed in mv."""
                mv = small.tile([P, NT, 2], f32, tag="mv")
                nsub = 2
                step = HW // nsub
                for t in range(NT):
                    st6 = small.tile([P, nsub, 6], f32, tag="st6")
                    for q in range(nsub):
                        iq = nc.vector.bn_stats(
                            out=st6[:, q, :],
                            in_=x_sb[:, b, t, q * step:(q + 1) * step])
                        if after is not None:
                            tile.add_dep_helper(iq.ins, after.ins, sync=False,
                                                reason="gn stats after prev apply")
                    nc.vector.bn_aggr(out=mv[:, t, :], in_=st6[:, :, :])
                msq = small.tile([P, NT], f32, tag="msq")
                nc.vector.tensor_mul(msq[:, :], mv[:, :, 0], mv[:, :, 0])
                nc.vector.tensor_add(mv[:, :, 1], mv[:, :, 1], msq[:, :])
                return mv

            def gn_affine(b, mv, use_act_sqrt=False):
                """fused group-avg+broadcast matmul, then form per-channel A/B."""
                bc_ps = psq.tile([P, 512], f32, tag="qkv")
                nc.tensor.matmul(bc_ps[:, :NT * 2], gg_sb[:, :], mv[:, :, :],
                                 start=True, stop=True)
                bc = small.tile([P, NT, 2], f32, tag="bcs")
                nc.vector.tensor_copy(bc[:, :, :], bc_ps[:, 0:NT * 2])
                vb = small.tile([P, NT], f32, tag="vb")
                nc.vector.tensor_mul(vb[:, :], bc[:, :, 0], bc[:, :, 0])
                nc.vector.tensor_sub(vb[:, :], bc[:, :, 1], vb[:, :])
                if use_act_sqrt:
                    # sample 0 (pre-exp): the shorter ACT chain wins and its
                    # Sqrt table load cannot evict a not-yet-loaded Exp table
                    nc.vector.tensor_scalar_add(vb[:, :], vb[:, :], EPS)
                    nc.scalar.sqrt(vb[:, :], vb[:, :])
                    nc.vector.reciprocal(vb[:, :], vb[:, :])
                    y1 = vb
                else:
                    nc.vector.tensor_scalar_add(vb[:, :], vb[:, :], EPS)
                    # rstd = rsqrt(var+eps): fast-inverse-sqrt + 2 Newton steps
                    # (all-DVE: keeps Sqrt off ACT so it never evicts Exp)
                    ii = small.tile([P, NT], mybir.dt.int32, tag="ii")
                    nc.vector.tensor_scalar(
                        out=ii[:, :], in0=vb.bitcast(mybir.dt.int32)[:, :],
                        scalar1=1, scalar2=None, op0=ALU.arith_shift_right)
                    nc.vector.tensor_tensor(ii[:, :], magic_sb[:, :], ii[:, :],
                                            op=ALU.subtract)
                    y0 = ii.bitcast(f32)
                    yt = small.tile([P, NT], f32, tag="yt")
                    y1 = small.tile([P, NT], f32, tag="y1")
                    nc.vector.tensor_mul(yt[:, :], vb[:, :], y0[:, :])
                    nc.vector.tensor_mul(yt[:, :], yt[:, :], y0[:, :])
                    nc.vector.tensor_scalar(out=yt[:, :], in0=yt[:, :], scalar1=-0.5,
                                            scalar2=1.5, op0=ALU.mult, op1=ALU.add)
                    nc.vector.tensor_mul(y1[:, :], y0[:, :], yt[:, :])
                    nc.vector.tensor_mul(yt[:, :], vb[:, :], y1[:, :])
                    nc.vector.tensor_mul(yt[:, :], yt[:, :], y1[:, :])
                    nc.vector.tensor_scalar(out=yt[:, :], in0=yt[:, :], scalar1=-0.5,
                                            scalar2=1.5, op0=ALU.mult, op1=ALU.add)
                    nc.vector.tensor_mul(y1[:, :], y1[:, :], yt[:, :])
                tmp = small.tile([P, NT], f32, tag="tmpab")
                nc.vector.tensor_mul(a_all[:, b, :], y1[:, :], gab_sb[:, 0, :])
                nc.vector.tensor_mul(tmp[:, :], bc[:, :, 0], a_all[:, b, :])
                nc.vector.tensor_sub(b_all[:, b, :], gab_sb[:, 1, :], tmp[:, :])

            def apply_h(b, after=None):
                """h = x*A + B (fp8)"""
                h = hp.tile([P, NT, HW], f8, tag="h")
                last = None
                for t in range(NT):
                    last = nc.vector.tensor_scalar(
                        out=h[:, t, :], in0=x_sb[:, b, t, :],
                        scalar1=a_all[:, b, t:t + 1], scalar2=b_all[:, b, t:t + 1],
                        op0=ALU.mult, op1=ALU.add,
                    )
                    if after is not None:
                        tile.add_dep_helper(last.ins, after.ins, sync=False,
                                            reason="apply after O-scales")
                return h, last

            def qk(b, h_sb):
                """Q,K fp8 [d, hw] via DoubleRow fp8 matmuls; psum->sbuf
                copies on DVE."""
                q_sb = qkp.tile([P, NT, HW], f8, tag="q")
                k_sb = qkp.tile([P, NT, HW], f8, tag="k")
                # Q psum->sbuf copies on DVE, K copies on ACT: split across
                # engines so the psq pool drains at 2x copy rate and neither
                # engine's queue backs up.
                for dst_sb, w_sb, bias_idx, eng in (
                        (q_sb, wq_sb, 0, "dve"), (k_sb, wk_sb, 1, "act")):
                    for dm in range(NT):
                        ps0 = psq.tile([P, 512], f32, tag="qkv")
                        ps1 = psq.tile([P, 512], f32, tag="qkv")
                        for kp in range(NP):
                            lhsT = w_sb[:, 2 * kp:2 * kp + 2, dm * P:(dm + 1) * P]
                            st, sp = (kp == 0), (kp == NP - 1)
                            nc.tensor.matmul(ps0[:, :], lhsT,
                                             h_sb[:, 2 * kp:2 * kp + 2, 0:512],
                                             start=st, stop=sp, perf_mode=DR)
                            nc.tensor.matmul(ps1[:, :], lhsT,
                                             h_sb[:, 2 * kp:2 * kp + 2, 512:1024],
                                             start=st, stop=sp, perf_mode=DR)
                        for ps, nsl in ((ps0, slice(0, 512)), (ps1, slice(512, 1024))):
                            if zero_qk_bias:
                                if eng == "act":
                                    nc.scalar.copy(dst_sb[:, dm, nsl], ps[:, :])
                                else:
                                    nc.vector.tensor_copy(dst_sb[:, dm, nsl], ps[:, :])
                            else:
                                if eng == "act":
                                    nc.scalar.add(dst_sb[:, dm, nsl], ps[:, :],
                                                  bqkt_sb[:, bias_idx, dm:dm + 1])
                                else:
                                    nc.vector.tensor_scalar(
                                        out=dst_sb[:, dm, nsl], in0=ps[:, :],
                                        scalar1=bqkt_sb[:, bias_idx, dm:dm + 1],
                                        scalar2=None, op0=ALU.add)
                return q_sb, k_sb

            def vmm(b, h_sb):
                """V bf16 transposed [hw, d]; psum->sbuf copies on ACT."""
                vt_sb = vtp.tile([P, NJ, C], bf16, tag="vt")
                for jm in range(NJ):
                    ps = psq.tile([P, 512], f32, tag="qkv")
                    for kp in range(NP):
                        nc.tensor.matmul(
                            ps[:, :],
                            h_sb[:, 2 * kp:2 * kp + 2, jm * P:(jm + 1) * P],
                            wv_sb[:, 2 * kp:2 * kp + 2, :],
                            start=(kp == 0), stop=(kp == NP - 1), perf_mode=DR,
                        )
                    if zero_v_bias:
                        nc.scalar.copy(vt_sb[:, jm, :], ps[:, :])
                    else:
                        nc.vector.tensor_add(vt_sb[:, jm, :], ps[:, :], bv_sb[:, :])
                return vt_sb

            # ---------------- prologue: GN(0), qk(0), GN(1) ----------------
            # V(0) is NOT issued here: its ACT copies would queue ahead of
            # exp(0) and delay the whole pipeline. It is issued inside
            # iteration 0 (after the den section), mirroring the steady-state
            # vmm(b+1) position, so its copies run after exp(0) drains.
            mv0 = gn_stats(0)
            gn_affine(0, mv0)
            h0_sb, last_apply = apply_h(0)
            h_next = h0_sb

            qk_next = qk(0, h0_sb)

            mv1 = gn_stats(1, after=last_apply)
            gn_affine(1, mv1)
            h_next, last_apply = apply_h(1)

            vt_next = None

            # ---------------- main loop ----------------
            for b in range(NB):
                q_sb, k_sb = qk_next
                vt_sb = vt_next
                h_sb = h_next

                # ---- scores transposed + exp (ACT) ----
                est_sb = estp.tile([P, NJ, HW], bf16, tag="est")
                for jm in range(NJ):
                    st_ps = psb.tile([P, HW], f32, tag="big")
                    for kp in range(NP):
                        lhsT = k_sb[:, 2 * kp:2 * kp + 2, jm * P:(jm + 1) * P]
                        st, sp = (kp == 0), (kp == NP - 1)
                        nc.tensor.matmul(st_ps[:, 0:512], lhsT,
                                         q_sb[:, 2 * kp:2 * kp + 2, 0:512],
                                         start=st, stop=sp, perf_mode=DR)
                        nc.tensor.matmul(st_ps[:, 512:1024], lhsT,
                                         q_sb[:, 2 * kp:2 * kp + 2, 512:1024],
                                         start=st, stop=sp, perf_mode=DR)
                    nc.scalar.activation(
                        out=est_sb[:, jm, :], in_=st_ps[:, :], func=ACT.Exp,
                        scale=CINV,
                    )

                # ---- softmax denominator ----
                # pairwise bf16 tree over the 8 exp tiles (DVE 2x rate), then
                # one (1/16)-matrix bf16 matmul reduces the last 128
                # partitions AND replicates den/16 across partitions. Issued
                # ahead of qk(b+1) so recip/rep unblocks PV's O-scales early.
                s0 = sump.tile([P, HW], bf16, tag="s0")
                s1 = sump.tile([P, HW], bf16, tag="s1")
                s2 = sump.tile([P, HW], bf16, tag="s2")
                s3 = sump.tile([P, HW], bf16, tag="s3")
                nc.vector.tensor_add(s0[:, :], est_sb[:, 0, :], est_sb[:, 1, :])
                nc.vector.tensor_add(s1[:, :], est_sb[:, 2, :], est_sb[:, 3, :])
                nc.vector.tensor_add(s2[:, :], est_sb[:, 4, :], est_sb[:, 5, :])
                nc.vector.tensor_add(s3[:, :], est_sb[:, 6, :], est_sb[:, 7, :])
                nc.vector.tensor_add(s0[:, :], s0[:, :], s1[:, :])
                nc.vector.tensor_add(s2[:, :], s2[:, :], s3[:, :])
                nc.vector.tensor_add(s0[:, :], s0[:, :], s2[:, :])

                # sample 0's V, deferred out of the prologue (see above)
                if b == 0:
                    vt_sb = vmm(0, h0_sb)

                # ---- Q,K for sample b+1 fill TensorE while ACT exps ----
                if b + 1 < NB:
                    qk_next = qk(b + 1, h_sb)

                # den's partition-reduce matmuls AFTER the qk/vmm matmuls in
                # the TensorE stream: they block on the DVE tree (which blocks
                # on exp), and rep is only needed by PV's first O-scale.
                rs0 = psq.tile([P, 512], f32, tag="qkv")
                rs1 = psq.tile([P, 512], f32, tag="qkv")
                nc.tensor.matmul(rs0[:, :], ones_sb[:, :], s0[:, 0:512],
                                 start=True, stop=True)
                nc.tensor.matmul(rs1[:, :], ones_sb[:, :], s0[:, 512:1024],
                                 start=True, stop=True)
                rep = op.tile([P, HW], f32, tag="rep")
                nc.vector.reciprocal_approx_fast(out=rep[:, 0:512], in_=rs0[:, :])
                nc.vector.reciprocal_approx_fast(out=rep[:, 512:1024], in_=rs1[:, :])

                # ---- PV (bf16): O16[c, i] = (sum_j VT[j, c] * est[j, i]) * rep ----
                o_sb = op.tile([P, NT, HW], f8, tag="o")
                for cm in range(NT):
                    o_ps = psb.tile([P, HW], f32, tag="big")
                    for jm in range(NJ):
                        lhsT = vt_sb[:, jm, cm * P:(cm + 1) * P]
                        st, sp = (jm == 0), (jm == NJ - 1)
                        nc.tensor.matmul(o_ps[:, 0:512], lhsT,
                                         est_sb[:, jm, 0:512], start=st, stop=sp)
                        nc.tensor.matmul(o_ps[:, 512:1024], lhsT,
                                         est_sb[:, jm, 512:1024], start=st, stop=sp)
                    last_oscale = nc.vector.tensor_mul(
                        o_sb[:, cm, :], o_ps[:, :], rep[:, :])

                # ---- V for sample b+1 (between PV and proj: its ACT copies
                #      run right after exp(b) ends, before exp(b+1) needs ACT)
                if b + 1 < NB:
                    vt_next = vmm(b + 1, h_sb)

                # ---- out-proj (fp8 DR on 16x o) + 1/16 + residual ----
                for dm in range(NT):
                    p_ps = psb.tile([P, HW], f32, tag="big")
                    for kp in range(NP):
                        lhsT = wt_sb[:, 2 * kp:2 * kp + 2, dm * P:(dm + 1) * P]
                        st, sp = (kp == 0), (kp == NP - 1)
                        nc.tensor.matmul(p_ps[:, 0:512], lhsT,
                                         o_sb[:, 2 * kp:2 * kp + 2, 0:512],
                                         start=st, stop=sp, perf_mode=DR)
                        nc.tensor.matmul(p_ps[:, 512:1024], lhsT,
                                         o_sb[:, 2 * kp:2 * kp + 2, 512:1024],
                                         start=st, stop=sp, perf_mode=DR)
                    out_t = outp.tile([P, HW], f32, tag="out")
                    if b == NB - 1:
                        # quarter-split the final residuals so the last
                        # out-DMAs start as early as possible
                        qengs = (nc.sync, nc.gpsimd, nc.scalar, nc.sync)
                        for qq in range(4):
                            h0 = qq * 256
                            if zero_t_bias:
                                nc.vector.scalar_tensor_tensor(
                                    out=out_t[:, h0:h0 + 256],
                                    in0=p_ps[:, h0:h0 + 256],
                                    scalar=1.0 / OSC,
                                    in1=x_sb[:, b, dm, h0:h0 + 256],
                                    op0=ALU.mult, op1=ALU.add,
                                )
                            else:
                                nc.vector.tensor_scalar(
                                    out=out_t[:, h0:h0 + 256],
                                    in0=p_ps[:, h0:h0 + 256],
                                    scalar1=1.0 / OSC,
                                    scalar2=bqkt_sb[:, 2, dm:dm + 1],
                                    op0=ALU.mult, op1=ALU.add)
                                nc.vector.tensor_add(
                                    out_t[:, h0:h0 + 256], out_t[:, h0:h0 + 256],
                                    x_sb[:, b, dm, h0:h0 + 256])
                            qengs[qq].dma_start(
                                out=out_d[b, dm * P:(dm + 1) * P, h0:h0 + 256],
                                in_=out_t[:, h0:h0 + 256])
                    elif dm < 2:
                        if zero_t_bias:
                            nc.vector.scalar_tensor_tensor(
                                out=out_t[:, :], in0=p_ps[:, :],
                                scalar=1.0 / OSC, in1=x_sb[:, b, dm, :],
                                op0=ALU.mult, op1=ALU.add,
                            )
                        else:
                            nc.vector.tensor_scalar(
                                out=out_t[:, :], in0=p_ps[:, :],
                                scalar1=1.0 / OSC,
                                scalar2=bqkt_sb[:, 2, dm:dm + 1],
                                op0=ALU.mult, op1=ALU.add)
                            nc.vector.tensor_add(
                                out_t[:, :], out_t[:, :], x_sb[:, b, dm, :])
                        nc.sync.dma_start(
                            out=out_d[b, dm * P:(dm + 1) * P, 0:512],
                            in_=out_t[:, 0:512])
                        nc.gpsimd.dma_start(
                            out=out_d[b, dm * P:(dm + 1) * P, 512:1024],
                            in_=out_t[:, 512:1024])
                    else:
                        # last two proj tiles: drain p_ps via an ACT copy
                        # (ACT is idle at the iteration boundary) so the psb
                        # psum buf frees for scores(b+1)'s first tiles without
                        # waiting behind the DVE residual queue; DVE then adds
                        # the residual from SBUF.
                        pt = outp.tile([P, HW], f32, tag="ptmp")
                        if zero_t_bias:
                            nc.scalar.mul(pt[:, :], p_ps[:, :], 1.0 / OSC)
                        else:
                            nc.scalar.activation(
                                out=pt[:, :], in_=p_ps[:, :],
                                func=ACT.Identity, scale=1.0 / OSC,
                                bias=bqkt_sb[:, 2, dm:dm + 1])
                        nc.vector.tensor_add(out_t[:, :], pt[:, :],
                                             x_sb[:, b, dm, :])
                        nc.sync.dma_start(
                            out=out_d[b, dm * P:(dm + 1) * P, 0:512],
                            in_=out_t[:, 0:512])
                        nc.gpsimd.dma_start(
                            out=out_d[b, dm * P:(dm + 1) * P, 512:1024],
                            in_=out_t[:, 512:1024])

                # sample b+2's GN, issued AFTER proj so the residuals (which
                # free the psb psum bufs that scores(b+1) needs) keep DVE
                # priority; the GN chain then fills early-scores(b+1) DVE
                # idle. apply is still ordered behind this sample's O-scales.
                if b + 2 < NB:
                    mv_next = gn_stats(b + 2, after=last_apply)
                    gn_affine(b + 2, mv_next)
                    h_next, last_apply = apply_h(b + 2, after=last_oscale)

    nc.compile()
    return nc


def prep_inputs(inputs):
    """Host-side prep: per-core in_maps with pre-laid-out weights/constants."""
    e4 = ml_dtypes.float8_e4m3
    x = np.ascontiguousarray(np.asarray(inputs["x"], dtype=np.float32)).reshape(
        B_FULL, C, HW
    )

    def wprep(w):
        # [C, C] -> [P, NT, C]  (lhsT slices w[kc*128+p, d])
        return np.ascontiguousarray(
            np.asarray(w, dtype=np.float32).reshape(NT, P, C).transpose(1, 0, 2)
        ).astype(e4)

    def cols(v):
        # [C] -> [P, NT]
        return np.ascontiguousarray(
            np.asarray(v, dtype=np.float32).reshape(NT, P).T
        )

    bqkt = np.stack([cols(inputs["bq"]), cols(inputs["bk"]), cols(inputs["bt"])],
                    axis=1)  # [P, 3, NT]
    gab = np.stack([cols(inputs["gn_scale"]), cols(inputs["gn_bias"])], axis=1)
    bv_rep = np.tile(np.asarray(inputs["bv"], dtype=np.float32)[None, :], (P, 1))
    gg = np.zeros((P, P), np.float32)
    for p in range(P):
        gg[p, (p // GS) * GS:(p // GS + 1) * GS] = 1.0 / GS

    shared = {
        "wq": wprep(inputs["Wq"]), "wk": wprep(inputs["Wk"]),
        "wv": wprep(inputs["Wv"]), "wt": wprep(inputs["Wt"]),
        "bqkt": np.ascontiguousarray(bqkt), "bv_rep": bv_rep,
        "gn_ab": np.ascontiguousarray(gab), "gg": gg,
    }
    in_maps = []
    for c_id in range(N_CORES):
        m = dict(shared)
        m["x"] = np.ascontiguousarray(x[c_id * NB:(c_id + 1) * NB])
        in_maps.append(m)
    return in_maps


_NC_CACHE = {}


def get_nc(zero_qk_bias=True, zero_v_bias=True, zero_t_bias=True):
    key = (zero_qk_bias, zero_v_bias, zero_t_bias)
    if key not in _NC_CACHE:
        _NC_CACHE[key] = build_nc(zero_qk_bias=zero_qk_bias,
                                  zero_v_bias=zero_v_bias,
                                  zero_t_bias=zero_t_bias)
    return _NC_CACHE[key]


def run(inputs, trace=False):
    from concourse.bass_utils import run_bass_kernel_spmd

    zq = bool(
        np.all(np.asarray(inputs["bq"]) == 0) and np.all(np.asarray(inputs["bk"]) == 0)
    )
    zv = bool(np.all(np.asarray(inputs["bv"]) == 0))
    zt = bool(np.all(np.asarray(inputs["bt"]) == 0))
    nc = get_nc(zero_qk_bias=zq, zero_v_bias=zv, zero_t_bias=zt)
    in_maps = prep_inputs(inputs)
    res = run_bass_kernel_spmd(
        nc, in_maps, core_ids=list(range(N_CORES)), trace=trace
    )
    out = np.concatenate([np.asarray(r["out"]) for r in res.results], axis=0)
    return out.reshape(B_FULL, C, H, W), res


def kernel(**inputs):
    out, _ = run(inputs, trace=False)
    return out
